# revision 17
# baseline (speedup 1.0000x reference)
"""Cross-attention kernel for Trainium2, sharded over 8 NeuronCores.

Problem (per reference):
  q = wq @ x_q + bq ; k = wk @ x_kv + bk ; v = wv @ x_kv + bv   (1x1 convs)
  per head: attn = softmax(q^T k / sqrt(hd)) ; out = attn @ v^T
  y = wo @ out + bo

Sharding: core c -> (batch b = c // 4, head n = c % 4). Each core runs one
head's full attention and produces the partial output projection
y_part = wo[:, head] @ out_head; the host sums the 4 head partials per batch.

Mathematically exact simplifications (as in the previous version):
  * bk drops (per-query logit shift cancels in softmax); bv folds into the
    host-side bias (softmax rows sum to 1); scale folds into wq/bq;
    no max-subtraction (logits ~N(0,1)); softmax denominator comes from a
    ones-column in the AV stationary; normalization deferred to the host.

This version's speed structure (target: the scalar engine's exp stream is
the only pacer, ~1.08us per [128,1024] logit tile):
  * QK^T runs in fp8 DoubleRow perf mode at 0.5 cycles/row (2x bf16) with a
    residual-pair trick that keeps bf16-class accuracy: q ~ q8 + r8 and
    k ~ k8 + s8 (each fp8 e4m3 plus its fp8 residual); one DoubleRow matmul
    with moving planes (q8;r8),(r8;q8) and stationary planes (k8;k8),(s8;s8)
    contracts all four cross terms, computing (k8+s8)^T (q8+r8) exactly.
  * Input DMAs are priority-ordered: the first 512 xkv columns and first
    1024 xq columns (plus wk/wq/bq at the head of the sync ring) land first,
    so the first exp fires ~17us instead of ~29us.
  * k/v/q projections are woven into the chunk-0 QK/exp stream (PE executes
    in order; each woven projection rides the exp-paced PSUM ring).
  * v^T is produced by a dense v projection (PSUM drained by the otherwise
    idle gpsimd engine) plus one hardware DMA transpose on the vector ring
    (which carries no input traffic, so no queueing behind the 4MB of
    activations).
  * Projection PSUM drains quantize to fp8 on the fly: DVE does the low
    halves (q8 low / r8 low, k8/s8 full-width), gpsimd the high halves.
  * The AV stationary is trimmed to 65 columns (64 v^T + ones), shortening
    every AV weight load.
"""

import numpy as np
import ml_dtypes

import concourse.bacc as bacc
import concourse.mybir as mybir
import concourse.tile as tile
from concourse.bass_utils import run_bass_kernel_spmd

F32 = mybir.dt.float32
BF16 = mybir.dt.bfloat16
FP8 = mybir.dt.float8e4
DR = mybir.MatmulPerfMode.DoubleRow
SUB = mybir.AluOpType.subtract
ADD = mybir.AluOpType.add

B, C, HGT, WID = 2, 256, 64, 64
S = HGT * WID  # 4096 pixels
NH, HD = 4, 64
NCORES = 8
P = 128
IC = 1024  # i-chunk width (2 PSUM banks)
NI = S // IC  # 4
NJ = S // P  # 32 j-blocks
SCALE = HD ** -0.5
KPRI = 512   # priority xkv columns (first k/v projection slice)
QPRI = 1024  # priority xq columns (chunk-0 q projection)
PRE = 5      # chunk-0 exps banked ahead of the first AV (covers the
             # first v-transpose piece without stalling the scalar engine)


def _emit(tc):
    nc = tc.nc
    xq = nc.dram_tensor("xq", [2, P, S], BF16, kind="ExternalInput").ap()
    xkv = nc.dram_tensor("xkv", [2, P, S], BF16, kind="ExternalInput").ap()
    # wqT/wkT carry duplicated columns (w^T | w^T) so the projection writes
    # both partition halves of PSUM with identical values; the fp8 quantize
    # + residual drains are then single full-width engine ops per plane.
    wqT = nc.dram_tensor("wqT", [2, P, P], BF16, kind="ExternalInput").ap()
    wkT = nc.dram_tensor("wkT", [2, P, P], BF16, kind="ExternalInput").ap()
    wvT = nc.dram_tensor("wvT", [2, P, HD], BF16, kind="ExternalInput").ap()
    woT = nc.dram_tensor("woT", [HD, C], BF16, kind="ExternalInput").ap()
    bq = nc.dram_tensor("bq", [P, 1], F32, kind="ExternalInput").ap()
    y = nc.dram_tensor("y", [2, P, S], F32, kind="ExternalOutput").ap()
    yden = nc.dram_tensor("yden", [1, S], F32, kind="ExternalOutput").ap()

    with (
        tc.tile_pool(name="const", bufs=1) as cpool,
        tc.tile_pool(name="xp", bufs=1) as xpool,
        tc.tile_pool(name="qkv", bufs=1) as qpool,
        tc.tile_pool(name="es", bufs=10) as epool,
        tc.tile_pool(name="epi", bufs=2) as fpool,
        tc.tile_pool(name="ps", bufs=2, space="PSUM") as pp,
    ):
        # ---- critical-path weights at the head of the sync HWDGE ring ----
        wq_sb = cpool.tile([P, 2 * P], BF16)
        wk_sb = cpool.tile([P, 2 * P], BF16)
        bq_sb = cpool.tile([P, 1], F32)
        for cch in range(2):
            nc.sync.dma_start(wk_sb[:, cch * P:(cch + 1) * P], wkT[cch])
        for cch in range(2):
            nc.sync.dma_start(wq_sb[:, cch * P:(cch + 1) * P], wqT[cch])
        nc.sync.dma_start(bq_sb[:], bq)

        # ---- activations: priority slices first, then the rest in pieces
        # (piece-granular DMAs so projections wait only on their own slice)
        xq_sb = [xpool.tile([P, S], BF16, tag=f"xq{i}", name=f"xq_sb{i}")
                 for i in range(2)]
        xkv_sb = [xpool.tile([P, S], BF16, tag=f"xkv{i}", name=f"xkv_sb{i}")
                  for i in range(2)]
        # scalar ring gets ONLY the two priority pieces: each dma_start
        # issued on the scalar ring costs ~0.7us of the ACT sequencer, and
        # ACT time is the kernel's pacer. Everything else rides the sync
        # ring or the gpsimd SWDGE queue (self-issued, off the ACT path).
        nc.scalar.dma_start(xkv_sb[1][:, 0:KPRI], xkv[1][:, 0:KPRI])
        nc.scalar.dma_start(xq_sb[1][:, 0:QPRI], xq[1][:, 0:QPRI])
        nc.sync.dma_start(xkv_sb[0][:, 0:KPRI], xkv[0][:, 0:KPRI])
        nc.sync.dma_start(xq_sb[0][:, 0:QPRI], xq[0][:, 0:QPRI])
        for s in range(1, 5):
            sl = slice(s * 512, (s + 1) * 512)
            nc.sync.dma_start(xkv_sb[0][:, sl], xkv[0][:, sl])

        # ---- non-critical weights + second-half activations on SWDGE ----
        wv_sb = cpool.tile([P, 2 * HD], BF16)
        for cch in range(2):
            nc.gpsimd.dma_start(wv_sb[:, cch * HD:(cch + 1) * HD], wvT[cch])
        wo_sb = cpool.tile([HD, C], BF16)
        nc.gpsimd.dma_start(wo_sb[:], woT)
        for s in range(1, 5):
            sl = slice(s * 512, (s + 1) * 512)
            nc.gpsimd.dma_start(xkv_sb[1][:, sl], xkv[1][:, sl])
        for s in range(5, S // 512):
            sl = slice(s * 512, (s + 1) * 512)
            nc.gpsimd.dma_start(xkv_sb[1][:, sl], xkv[1][:, sl])
            nc.gpsimd.dma_start(xkv_sb[0][:, sl], xkv[0][:, sl])
        for t in range(1, S // 1024):
            sl = slice(t * 1024, (t + 1) * 1024)
            nc.gpsimd.dma_start(xq_sb[1][:, sl], xq[1][:, sl])

        # Zero bias for exp via memset (a float bias would become a DMA'd
        # const tensor queued behind the input DMAs).
        zbias_sb = cpool.tile([P, 1], F32)
        nc.vector.memset(zbias_sb[:], 0.0)

        # PE warmup burst: dense matmuls on scratch data while the input
        # DMAs are in flight. The activity monitor promotes the PE to
        # 2.4GHz after ~3.4us of sustained streaming; this keeps the PE
        # warm up to the first projection (~11us in).
        wrm_sb = cpool.tile([P, 512], BF16)
        nc.vector.memset(wrm_sb[:], 0.0)
        for w in range(18):
            wp = pp.tile([P, 512], F32, tag="st", bufs=2, name="wp")
            nc.tensor.matmul(wp[:], wrm_sb[:, 0:P], wrm_sb[:],
                             start=True, stop=True)
        # Warmup exp so the ~2.7us activation-table load happens before the
        # first real exp.
        warm_sb = cpool.tile([P, 1], BF16)
        nc.scalar.activation(warm_sb[:], zbias_sb[:],
                             mybir.ActivationFunctionType.Exp,
                             bias=zbias_sb[:])

        # ---- fp8 operand tiles ----
        # Moving planes [p, t, i]: t0 = (q8; r8), t1 = (r8; q8)
        x8 = qpool.tile([P, 2 * S], FP8)
        x8v = x8.rearrange("p (t n) -> p t n", t=2)
        # Stationary planes [p, t, j]: t0 = (k8; k8), t1 = (s8; s8)
        w8 = qpool.tile([P, 2 * S], FP8)
        w8v = w8.rearrange("p (t n) -> p t n", t=2)
        # v (dense, pre-transpose); rows 0:64 only
        v_sb = qpool.tile([HD, S], BF16)
        # v^T blocks [j-part, (block, 128)]: cols 0:64 = v^T (transpose
        # target; the hardware transpose requires this 128-stride block
        # layout — a 65-stride dest writes garbage), col 64 = ones. The AV
        # stationary slices only cols 0:65, so cols 65:128 stay untouched.
        va_sb = qpool.tile([P, NJ * P], BF16)
        va_v = va_sb.rearrange("p (j c) -> p j c", c=P)
        nc.vector.memset(va_v[:, :, HD:HD + 1], 1.0)

        # ---- projections ----
        def k_proj(s):
            sl = slice(s * 512, (s + 1) * 512)
            kp = pp.tile([P, 512], F32, tag="st", bufs=2, name="kp")
            nc.tensor.matmul(kp[:], wk_sb[:, 0:P], xkv_sb[0][:, sl],
                             start=True, stop=False)
            nc.tensor.matmul(kp[:], wk_sb[:, P:2 * P], xkv_sb[1][:, sl],
                             start=False, stop=True)
            # k8 then s8 = fp8(k - k8); kp holds (k; k) so both planes are
            # single full-width ops
            nc.vector.tensor_copy(w8v[:, 0, sl], kp[:])
            nc.vector.tensor_tensor(w8v[:, 1, sl], kp[:], w8v[:, 0, sl], SUB)

        def q_proj(t, act_assist=False):
            sl = slice(t * 512, (t + 1) * 512)
            qp = pp.tile([P, 512], F32, tag="st", bufs=2, name="qp")
            nc.tensor.matmul(qp[:], wq_sb[:, 0:P], xq_sb[0][:, sl],
                             start=True, stop=False)
            nc.tensor.matmul(qp[:], wq_sb[:, P:2 * P], xq_sb[1][:, sl],
                             start=False, stop=True)
            # qp holds (q; q); bias added during the drains. For the two
            # prologue slices the q8 planes run on the (pre-exp-idle) scalar
            # engine so the DVE chain isn't the first-exp critical path.
            LO, HI = slice(0, HD), slice(HD, P)
            if act_assist:
                nc.scalar.activation(x8v[LO, 0, sl], qp[LO, :],
                                     mybir.ActivationFunctionType.Identity,
                                     bias=bq_sb[LO])
                nc.scalar.activation(x8v[HI, 1, sl], qp[HI, :],
                                     mybir.ActivationFunctionType.Identity,
                                     bias=bq_sb[HI])
            else:
                nc.vector.tensor_scalar_add(x8v[LO, 0, sl], qp[LO, :],
                                            bq_sb[LO])
                nc.vector.tensor_scalar_add(x8v[HI, 1, sl], qp[HI, :],
                                            bq_sb[HI])
            nc.vector.scalar_tensor_tensor(
                x8v[LO, 1, sl], qp[LO, :], bq_sb[LO], x8v[LO, 0, sl], ADD, SUB)
            nc.vector.scalar_tensor_tensor(
                x8v[HI, 0, sl], qp[HI, :], bq_sb[HI], x8v[HI, 1, sl], ADD, SUB)

        def v_proj(s):
            sl = slice(s * 512, (s + 1) * 512)
            vp = pp.tile([HD, 512], F32, tag="av", bufs=2, name="vp")
            nc.tensor.matmul(vp[:], wv_sb[:, 0:HD], xkv_sb[0][:, sl],
                             start=True, stop=False)
            nc.tensor.matmul(vp[:], wv_sb[:, HD:2 * HD], xkv_sb[1][:, sl],
                             start=False, stop=True)
            nc.vector.tensor_copy(v_sb[:, sl], vp[:])

        k_proj(0)
        q_proj(0, act_assist=True)
        q_proj(1, act_assist=True)
        v_proj(0)

        # ---- attention ----
        def qk_exp(c, j):
            st = pp.tile([P, IC], F32, tag="st", bufs=2, name="st")
            for h in range(IC // 512):
                isl = slice(c * IC + h * 512, c * IC + (h + 1) * 512)
                nc.tensor.matmul(st[:, h * 512:(h + 1) * 512],
                                 w8v[:, :, j * P:(j + 1) * P],
                                 x8v[:, :, isl],
                                 start=True, stop=True, perf_mode=DR)
            et = epool.tile([P, IC], BF16, name="et")
            nc.scalar.activation(et[:], st[:],
                                 mybir.ActivationFunctionType.Exp,
                                 bias=zbias_sb[:])
            return et

        # Softmax normalization is deferred to the host: the device ships
        # y_un = wo_col @ (exp(S^T)^T V)^T plus per-pixel denominators.
        pend = [None] * NI

        def epilogue_part2(i, final=False, ohs=(0, 1)):
            outt = pend[i]
            for oh in ohs:
                for h in range(IC // 512):
                    yp = pp.tile([P, 512], F32, tag="av", bufs=2, name="yp")
                    nc.tensor.matmul(yp[:], wo_sb[:, oh * P:(oh + 1) * P],
                                     outt[:, h * 512:(h + 1) * 512],
                                     start=True, stop=True)
                    ys = fpool.tile([P, 512], F32, name="ys")
                    if final and (oh + h) % 2 == 1:
                        nc.scalar.activation(
                            ys[:], yp[:], mybir.ActivationFunctionType.Copy)
                    else:
                        nc.vector.tensor_copy(ys[:], yp[:])
                    eng = nc.sync if oh == 0 else nc.scalar
                    eng.dma_start(
                        y[oh][:, i * IC + h * 512:i * IC + (h + 1) * 512],
                        ys[:])

        # Chunk-0 weave: remaining projections ride the exp-paced stream at
        # at most one PE-pair per j — enough PE duty to hold the 2.4GHz
        # p-state through the bank phase, little enough not to starve the
        # exp stream. v projections use the (still unallocated) av-tag PSUM
        # ring; k/q share the st ring.
        def transpose_piece(g):
            # v^T for j-blocks 8g..8g+7, available as soon as v slices
            # 2g/2g+1 are drained — the first AV only needs piece 0.
            nc.sync.dma_start_transpose(
                out=va_v[:, 8 * g:8 * (g + 1), 0:HD],
                in_=v_sb[:, 1024 * g:1024 * (g + 1)])

        def xq0_rest():
            for t in range(1, S // 1024):
                sl = slice(t * 1024, (t + 1) * 1024)
                nc.sync.dma_start(xq_sb[0][:, sl], xq[0][:, sl])

        weave0 = {
            0: [lambda: v_proj(1)],
            1: [lambda: k_proj(1), lambda: transpose_piece(0)],
            2: [lambda: v_proj(2)],
            3: [lambda: v_proj(3), lambda: transpose_piece(1)],
            4: [lambda: k_proj(2)],
            5: [lambda: v_proj(4)],
            6: [lambda: v_proj(5), lambda: transpose_piece(2)],
            7: [lambda: k_proj(3)],
            8: [lambda: v_proj(6)],
            9: [lambda: v_proj(7)],
            10: [lambda: k_proj(4), lambda: transpose_piece(3),
                 lambda: xq0_rest()],
            12: [lambda: k_proj(5)],
            14: [lambda: k_proj(6)],
            16: [lambda: k_proj(7)],
            18: [lambda: q_proj(2)],
            20: [lambda: q_proj(3)],
            22: [lambda: q_proj(4)],
            24: [lambda: q_proj(5)],
            26: [lambda: q_proj(6)],
            28: [lambda: q_proj(7)],
        }

        bank = []
        for j in range(PRE):
            bank.append(qk_exp(0, j))
            for fn in weave0.get(j, []):
                fn()

        for i in range(NI):
            av = pp.tile([HD + 1, IC], F32, tag="av", bufs=2, name="av")
            for j in range(NJ):
                if i > 0 and j == 8:
                    epilogue_part2(i - 1, ohs=(0,))
                if i > 0 and j == 10:
                    epilogue_part2(i - 1, ohs=(1,))
                if i == 0 and j < PRE:
                    et = bank[j]
                else:
                    et = qk_exp(i, j)
                    if i == 0:
                        for fn in weave0.get(j, []):
                            fn()
                for h in range(IC // 512):
                    nc.tensor.matmul(av[:, h * 512:(h + 1) * 512],
                                     va_v[:, j, 0:HD + 1],
                                     et[:, h * 512:(h + 1) * 512],
                                     start=(j == 0), stop=(j == NJ - 1))

            outt = fpool.tile([HD, IC], BF16, name="outt")
            if i == NI - 1:
                nc.vector.tensor_copy(outt[:, 0:512], av[0:HD, 0:512])
                nc.vector.tensor_copy(outt[:, 512:IC], av[0:HD, 512:IC])
            else:
                nc.vector.tensor_copy(outt[:], av[0:HD, :])
            den = fpool.tile([1, IC], F32, name="den")
            nc.vector.tensor_copy(den[:], av[HD:HD + 1, :])
            nc.gpsimd.dma_start(yden[:, i * IC:(i + 1) * IC], den[:])
            pend[i] = outt

        epilogue_part2(NI - 1, final=True)


def build():
    nc = bacc.Bacc("TRN2", target_bir_lowering=False, debug=False,
                   enable_asserts=False)
    with tile.TileContext(nc) as tc:
        _emit(tc)
    nc.compile()
    return nc


_NC_CACHE = []


def _get_nc():
    if not _NC_CACHE:
        _NC_CACHE.append(build())
    return _NC_CACHE[0]


def make_in_maps(x_q, x_kv, wq, bq, wk, bk, wv, bv, wo, bo):
    bf = ml_dtypes.bfloat16
    in_maps = []
    bo_effs = []
    for c in range(NCORES):
        b, n = divmod(c, NH)
        hs = slice(n * HD, (n + 1) * HD)
        wq_h = wq[hs].astype(np.float64) * SCALE
        bo_eff = wo[:, hs].astype(np.float64) @ bv[hs].astype(np.float64)
        if n == 0:
            bo_eff = bo_eff + bo.astype(np.float64)
        bo_effs.append(bo_eff.astype(np.float32))
        wq_dup = np.concatenate([wq_h.T, wq_h.T], axis=1)  # [256, 128]
        wk_dup = np.concatenate([wk[hs].T, wk[hs].T], axis=1)
        bq_h = (bq[hs].astype(np.float64) * SCALE).astype(np.float32)
        in_maps.append({
            "xq": np.ascontiguousarray(
                x_q[b].reshape(C, S).reshape(2, P, S)).astype(bf),
            "xkv": np.ascontiguousarray(
                x_kv[b].reshape(C, S).reshape(2, P, S)).astype(bf),
            "wqT": np.ascontiguousarray(wq_dup.reshape(2, P, P)).astype(bf),
            "wkT": np.ascontiguousarray(wk_dup.reshape(2, P, P)).astype(bf),
            "wvT": np.ascontiguousarray(
                wv[hs].T.reshape(2, P, HD)).astype(bf),
            "woT": np.ascontiguousarray(wo[:, hs].T).astype(bf),
            "bq": np.tile(bq_h, 2).reshape(P, 1),
        })
    return in_maps, bo_effs


def assemble_output(results, bo_effs):
    # y_core is the unnormalized head partial; divide by the softmax
    # denominator and add the (host-folded) bias here.
    y = np.zeros((B, C, S), np.float32)
    for c in range(NCORES):
        b = c // NH
        den = results[c]["yden"].reshape(1, S)
        y[b] += results[c]["y"].reshape(C, S) / den \
            + bo_effs[c].reshape(C, 1)
    return y.reshape(B, C, HGT, WID)


def kernel(**inputs):
    nc = _get_nc()
    in_maps, bo_effs = make_in_maps(**inputs)
    res = run_bass_kernel_spmd(nc, in_maps, list(range(NCORES)))
    return assemble_output(res.results, bo_effs)


if __name__ == "__main__":
    nc = build()
    print("built + compiled ok")


# revision 18
# speedup vs baseline: 1.0707x; 1.0707x over previous
"""Cross-attention kernel for Trainium2, sharded over 8 NeuronCores.

Problem (per reference):
  q = wq @ x_q + bq ; k = wk @ x_kv + bk ; v = wv @ x_kv + bv   (1x1 convs)
  per head: attn = softmax(q^T k / sqrt(hd)) ; out = attn @ v^T
  y = wo @ out + bo

Sharding: core c -> (batch b = c // 4, head n = c % 4). Each core runs one
head's full attention and produces the partial output projection
y_part = wo[:, head] @ out_head; the host sums the 4 head partials per batch.

Mathematically exact simplifications (as in the 192us version):
  * bk drops (per-query logit shift cancels in softmax); bv folds into the
    host-side bias (softmax rows sum to 1); scale folds into wq/bq;
    no max-subtraction (logits ~N(0,1)); softmax denominator comes from a
    ones-column in the AV stationary; normalization deferred to the host
    (ships y_un + per-pixel denominators, host divides).

Speed structure. The scalar engine's exp stream is the pacer: 128 exp
instructions of [128,1024] at ~1.11us each (1 elem/cycle/partition at
1.2GHz + ~230ns instruction overhead) ~= 142us; everything else must hide
under it. Measured facts this schedule is built on (NTFF traces):
  * A 512-col matmul costs ~215ns streaming at the promoted 2.4GHz PE
    clock + ~100-135ns weight load; per attention iteration the PE runs
    QK(2) + AV(2) matmuls ~= 1.05-1.25us, just under the exp. fp8
    DoubleRow gives NO streaming speedup on this silicon (tried: 512-out
    DoubleRow measures ~375ns like bf16), so everything stays bf16.
  * The PE p-state: ~5us of continuous warmup matmuls promote 1.2->2.4GHz
    (~13us in, right before the first projection); the exp-paced stream's
    small per-iteration gaps then hold it.
  * Each early dma_start issued on the scalar ring costs ~3.5us of ACT
    sequencer time, so the scalar ring carries only late y-output DMAs;
    inputs ride the sync ring (priority slices first: first 512 xkv cols,
    first 1024 xq cols, weights at the head) and the gpsimd SWDGE queue.
  * The remaining k|v / q projections are woven into the chunk-0 stream at
    at most one matmul-pair per j (PE executes in order; heavier weaves
    starve the exp stream 1:1, lighter ones let the p-state drop).
  * v^T is produced per-1024-column piece: fused k|v projection, DVE
    drain, and four piece-granular hardware DMA transposes — the first AV
    only waits for piece 0, so only PRE=4 exps need banking and the
    end-of-stream AV debt (the kernel tail) stays small.
  * The AV stationary is trimmed to 65 columns (64 v^T + ones).
"""

import numpy as np
import ml_dtypes

import concourse.bacc as bacc
import concourse.mybir as mybir
import concourse.tile as tile
from concourse.bass_utils import run_bass_kernel_spmd

F32 = mybir.dt.float32
BF16 = mybir.dt.bfloat16

B, C, HGT, WID = 2, 256, 64, 64
S = HGT * WID  # 4096 pixels
NH, HD = 4, 64
NCORES = 8
P = 128
IC = 1024  # i-chunk width (2 PSUM banks)
NI = S // IC  # 4
NJ = S // P  # 32 j-blocks
SCALE = HD ** -0.5
KPRI = 512   # priority xkv columns (first k|v projection slice)
QPRI = 1024  # priority xq columns (chunk-0 q projection)
PRE = 4      # chunk-0 exps banked ahead of the first AV (covers the
             # first v-transpose piece)


def _emit(tc):
    nc = tc.nc
    xq = nc.dram_tensor("xq", [2, P, S], BF16, kind="ExternalInput").ap()
    xkv = nc.dram_tensor("xkv", [2, P, S], BF16, kind="ExternalInput").ap()
    wqT = nc.dram_tensor("wqT", [2, P, HD], BF16, kind="ExternalInput").ap()
    wkvT = nc.dram_tensor("wkvT", [2, P, P], BF16, kind="ExternalInput").ap()
    woT = nc.dram_tensor("woT", [HD, C], BF16, kind="ExternalInput").ap()
    bq = nc.dram_tensor("bq", [HD, 1], F32, kind="ExternalInput").ap()
    y = nc.dram_tensor("y", [2, P, S], F32, kind="ExternalOutput").ap()
    yden = nc.dram_tensor("yden", [1, S], F32, kind="ExternalOutput").ap()

    with (
        tc.tile_pool(name="const", bufs=1) as cpool,
        tc.tile_pool(name="xp", bufs=1) as xpool,
        tc.tile_pool(name="qkv", bufs=1) as qpool,
        tc.tile_pool(name="es", bufs=10) as epool,
        tc.tile_pool(name="epi", bufs=2) as fpool,
        tc.tile_pool(name="ps", bufs=2, space="PSUM") as pp,
    ):
        # ---- critical-path weights + priority slices head the sync ring --
        wkv_sb = cpool.tile([P, 2 * P], BF16)
        wq_sb = cpool.tile([P, 2 * HD], BF16)
        bq_sb = cpool.tile([HD, 1], F32)
        for cch in range(2):
            nc.sync.dma_start(wkv_sb[:, cch * P:(cch + 1) * P], wkvT[cch])
        for cch in range(2):
            nc.sync.dma_start(wq_sb[:, cch * HD:(cch + 1) * HD], wqT[cch])
        nc.sync.dma_start(bq_sb[:], bq)

        xq_sb = [xpool.tile([P, S], BF16, tag=f"xq{i}", name=f"xq_sb{i}")
                 for i in range(2)]
        xkv_sb = [xpool.tile([P, S], BF16, tag=f"xkv{i}", name=f"xkv_sb{i}")
                  for i in range(2)]
        # Inputs ride sync + SWDGE only: early dma_starts on the scalar
        # ring cost ~3.5us of ACT sequencer each, straight off the pacer.
        nc.sync.dma_start(xkv_sb[1][:, 0:KPRI], xkv[1][:, 0:KPRI])
        nc.sync.dma_start(xkv_sb[0][:, 0:KPRI], xkv[0][:, 0:KPRI])
        nc.sync.dma_start(xq_sb[1][:, 0:QPRI], xq[1][:, 0:QPRI])
        nc.sync.dma_start(xq_sb[0][:, 0:QPRI], xq[0][:, 0:QPRI])
        for s in range(1, 5):
            sl = slice(s * 512, (s + 1) * 512)
            nc.sync.dma_start(xkv_sb[0][:, sl], xkv[0][:, sl])

        # ---- the rest on the gpsimd SWDGE queue (self-issued) ----
        wo_sb = cpool.tile([HD, C], BF16)
        nc.gpsimd.dma_start(wo_sb[:], woT)
        for s in range(1, 5):
            sl = slice(s * 512, (s + 1) * 512)
            nc.gpsimd.dma_start(xkv_sb[1][:, sl], xkv[1][:, sl])
        for s in range(5, S // 512):
            sl = slice(s * 512, (s + 1) * 512)
            nc.gpsimd.dma_start(xkv_sb[1][:, sl], xkv[1][:, sl])
            nc.gpsimd.dma_start(xkv_sb[0][:, sl], xkv[0][:, sl])
        for t in range(1, S // 1024):
            sl = slice(t * 1024, (t + 1) * 1024)
            nc.gpsimd.dma_start(xq_sb[1][:, sl], xq[1][:, sl])

        # Zero bias for exp via memset (a float bias would become a DMA'd
        # const tensor queued behind the input DMAs).
        zbias_sb = cpool.tile([P, 1], F32)
        nc.vector.memset(zbias_sb[:], 0.0)

        # PE warmup burst: ~10us of dense matmuls while the input DMAs are
        # in flight; the activity monitor promotes the PE to 2.4GHz after
        # ~5us of sustained streaming, right before the first projection.
        wrm_sb = cpool.tile([P, 512], BF16)
        nc.vector.memset(wrm_sb[:], 0.0)
        for w in range(16):
            wp = pp.tile([P, 512], F32, tag="st", bufs=2, name="wp")
            nc.tensor.matmul(wp[:], wrm_sb[:, 0:P], wrm_sb[:],
                             start=True, stop=True)
        # Warmup exp so the ~2.7us activation-table load happens before the
        # first real exp.
        warm_sb = cpool.tile([P, 1], BF16)
        nc.scalar.activation(warm_sb[:], zbias_sb[:],
                             mybir.ActivationFunctionType.Exp,
                             bias=zbias_sb[:])

        # q/k zero-padded to 128 partitions (the zero rows contribute
        # nothing to the contraction).
        q_sb = qpool.tile([P, S], BF16)
        k_sb = qpool.tile([P, S], BF16)
        nc.vector.memset(q_sb[HD:P, :], 0.0)
        nc.vector.memset(k_sb[HD:P, :], 0.0)
        # v (dense, pre-transpose) lives on partitions 64:128 (the fused
        # k|v projection's PSUM rows), ready for the hardware transpose.
        v_sb = qpool.tile([P, S], BF16)
        # v^T blocks [j-part, (block, 128)]: cols 0:64 = v^T (the hardware
        # transpose requires the 128-stride block layout), col 64 = ones.
        # The AV stationary slices only cols 0:65.
        va_sb = qpool.tile([P, NJ * P], BF16)
        va_v = va_sb.rearrange("p (j c) -> p j c", c=P)
        nc.vector.memset(va_v[:, :, HD:HD + 1], 1.0)

        # ---- projections ----
        def kv_proj(s):
            # fused: stationary (wk^T | wv^T) -> PSUM rows 0:64 = k,
            # rows 64:128 = v, one matmul pass per 512-column slice
            sl = slice(s * 512, (s + 1) * 512)
            kvp = pp.tile([P, 512], F32, tag="st", bufs=2, name="kvp")
            nc.tensor.matmul(kvp[:], wkv_sb[:, 0:P], xkv_sb[0][:, sl],
                             start=True, stop=False)
            nc.tensor.matmul(kvp[:], wkv_sb[:, P:2 * P], xkv_sb[1][:, sl],
                             start=False, stop=True)
            nc.vector.tensor_copy(k_sb[0:HD, sl], kvp[0:HD, :])
            nc.vector.tensor_copy(v_sb[HD:P, sl], kvp[HD:P, :])

        def q_proj(t):
            sl = slice(t * 512, (t + 1) * 512)
            qp = pp.tile([HD, 512], F32, tag="st", bufs=2, name="qp")
            nc.tensor.matmul(qp[:], wq_sb[:, 0:HD], xq_sb[0][:, sl],
                             start=True, stop=False)
            nc.tensor.matmul(qp[:], wq_sb[:, HD:2 * HD], xq_sb[1][:, sl],
                             start=False, stop=True)
            nc.vector.tensor_scalar_add(q_sb[0:HD, sl], qp[:], bq_sb[:])

        kv_proj(0)
        q_proj(0)
        q_proj(1)

        def transpose_piece(g):
            # v^T for j-blocks 8g..8g+7, available as soon as v slices
            # 2g/2g+1 are drained — the first AV only needs piece 0.
            nc.sync.dma_start_transpose(
                out=va_v[:, 8 * g:8 * (g + 1), 0:HD],
                in_=v_sb[HD:P, 1024 * g:1024 * (g + 1)])

        def xq0_rest():
            for t in range(1, S // 1024):
                sl = slice(t * 1024, (t + 1) * 1024)
                nc.sync.dma_start(xq_sb[0][:, sl], xq[0][:, sl])

        # ---- attention ----
        def qk_exp(c, j):
            st = pp.tile([P, IC], F32, tag="st", bufs=2, name="st")
            for h in range(IC // 512):
                isl = slice(c * IC + h * 512, c * IC + (h + 1) * 512)
                nc.tensor.matmul(st[:, h * 512:(h + 1) * 512],
                                 k_sb[:, j * P:(j + 1) * P],
                                 q_sb[:, isl],
                                 start=True, stop=True)
            et = epool.tile([P, IC], BF16, name="et")
            nc.scalar.activation(et[:], st[:],
                                 mybir.ActivationFunctionType.Exp,
                                 bias=zbias_sb[:])
            return et

        pend = [None] * NI

        def epilogue_part2(i, final=False, ohs=(0, 1)):
            outt = pend[i]
            for oh in ohs:
                for h in range(IC // 512):
                    yp = pp.tile([P, 512], F32, tag="av", bufs=2, name="yp")
                    nc.tensor.matmul(yp[:], wo_sb[:, oh * P:(oh + 1) * P],
                                     outt[:, h * 512:(h + 1) * 512],
                                     start=True, stop=True)
                    ys = fpool.tile([P, 512], F32, name="ys")
                    if final and (oh + h) % 2 == 1:
                        nc.scalar.activation(
                            ys[:], yp[:], mybir.ActivationFunctionType.Copy)
                    else:
                        nc.vector.tensor_copy(ys[:], yp[:])
                    eng = nc.sync if oh == 0 else nc.scalar
                    eng.dma_start(
                        y[oh][:, i * IC + h * 512:i * IC + (h + 1) * 512],
                        ys[:])

        # Chunk-0 weave: remaining projections + transpose pieces ride the
        # exp-paced stream at at most one matmul-pair per j.
        weave0 = {
            0: [lambda: kv_proj(1)],
            1: [lambda: transpose_piece(0)],
            2: [lambda: kv_proj(2)],
            4: [lambda: kv_proj(3)],
            5: [lambda: transpose_piece(1)],
            6: [lambda: kv_proj(4)],
            8: [lambda: kv_proj(5)],
            9: [lambda: transpose_piece(2)],
            10: [lambda: kv_proj(6)],
            12: [lambda: kv_proj(7)],
            13: [lambda: transpose_piece(3)],
            14: [lambda: xq0_rest()],
            15: [lambda: q_proj(2)],
            17: [lambda: q_proj(3)],
            19: [lambda: q_proj(4)],
            21: [lambda: q_proj(5)],
            23: [lambda: q_proj(6)],
            25: [lambda: q_proj(7)],
        }

        bank = []
        for j in range(PRE):
            bank.append(qk_exp(0, j))
            for fn in weave0.get(j, []):
                fn()

        for i in range(NI):
            av = pp.tile([HD + 1, IC], F32, tag="av", bufs=2, name="av")
            for j in range(NJ):
                if i > 0 and j == 8:
                    epilogue_part2(i - 1, ohs=(0,))
                if i > 0 and j == 10:
                    epilogue_part2(i - 1, ohs=(1,))
                if i == 0 and j < PRE:
                    et = bank[j]
                else:
                    et = qk_exp(i, j)
                    if i == 0:
                        for fn in weave0.get(j, []):
                            fn()
                for h in range(IC // 512):
                    nc.tensor.matmul(av[:, h * 512:(h + 1) * 512],
                                     va_v[:, j, 0:HD + 1],
                                     et[:, h * 512:(h + 1) * 512],
                                     start=(j == 0), stop=(j == NJ - 1))

            outt = fpool.tile([HD, IC], BF16, name="outt")
            if i == NI - 1:
                nc.vector.tensor_copy(outt[:, 0:512], av[0:HD, 0:512])
                nc.vector.tensor_copy(outt[:, 512:IC], av[0:HD, 512:IC])
            else:
                nc.vector.tensor_copy(outt[:], av[0:HD, :])
            den = fpool.tile([1, IC], F32, name="den")
            nc.vector.tensor_copy(den[:], av[HD:HD + 1, :])
            nc.gpsimd.dma_start(yden[:, i * IC:(i + 1) * IC], den[:])
            pend[i] = outt

        epilogue_part2(NI - 1, final=True)


def build():
    nc = bacc.Bacc("TRN2", target_bir_lowering=False, debug=False,
                   enable_asserts=False)
    with tile.TileContext(nc) as tc:
        _emit(tc)
    nc.compile()
    return nc


_NC_CACHE = []


def _get_nc():
    if not _NC_CACHE:
        _NC_CACHE.append(build())
    return _NC_CACHE[0]


def make_in_maps(x_q, x_kv, wq, bq, wk, bk, wv, bv, wo, bo):
    bf = ml_dtypes.bfloat16
    in_maps = []
    bo_effs = []
    for c in range(NCORES):
        b, n = divmod(c, NH)
        hs = slice(n * HD, (n + 1) * HD)
        wq_h = wq[hs].astype(np.float64) * SCALE
        bo_eff = wo[:, hs].astype(np.float64) @ bv[hs].astype(np.float64)
        if n == 0:
            bo_eff = bo_eff + bo.astype(np.float64)
        bo_effs.append(bo_eff.astype(np.float32))
        in_maps.append({
            "xq": np.ascontiguousarray(
                x_q[b].reshape(C, S).reshape(2, P, S)).astype(bf),
            "xkv": np.ascontiguousarray(
                x_kv[b].reshape(C, S).reshape(2, P, S)).astype(bf),
            "wqT": np.ascontiguousarray(wq_h.T.reshape(2, P, HD)).astype(bf),
            "wkvT": np.ascontiguousarray(
                np.concatenate([wk[hs].T, wv[hs].T], axis=1)
                .reshape(2, P, P)).astype(bf),
            "woT": np.ascontiguousarray(wo[:, hs].T).astype(bf),
            "bq": (bq[hs].astype(np.float64) * SCALE
                   ).astype(np.float32).reshape(HD, 1),
        })
    return in_maps, bo_effs


def assemble_output(results, bo_effs):
    # y_core is the unnormalized head partial; divide by the softmax
    # denominator and add the (host-folded) bias here.
    y = np.zeros((B, C, S), np.float32)
    for c in range(NCORES):
        b = c // NH
        den = results[c]["yden"].reshape(1, S)
        y[b] += results[c]["y"].reshape(C, S) / den \
            + bo_effs[c].reshape(C, 1)
    return y.reshape(B, C, HGT, WID)


def kernel(**inputs):
    nc = _get_nc()
    in_maps, bo_effs = make_in_maps(**inputs)
    res = run_bass_kernel_spmd(nc, in_maps, list(range(NCORES)))
    return assemble_output(res.results, bo_effs)


if __name__ == "__main__":
    nc = build()
    print("built + compiled ok")


# revision 19
# speedup vs baseline: 1.0754x; 1.0044x over previous
"""Cross-attention kernel for Trainium2, sharded over 8 NeuronCores.

Problem (per reference):
  q = wq @ x_q + bq ; k = wk @ x_kv + bk ; v = wv @ x_kv + bv   (1x1 convs)
  per head: attn = softmax(q^T k / sqrt(hd)) ; out = attn @ v^T
  y = wo @ out + bo

Sharding: core c -> (batch b = c // 4, head n = c % 4). Each core runs one
head's full attention and produces the partial output projection
y_part = wo[:, head] @ out_head; the host sums the 4 head partials per batch.

Mathematically exact simplifications (as in the 192us version):
  * bk drops (per-query logit shift cancels in softmax); bv folds into the
    host-side bias (softmax rows sum to 1); scale folds into wq/bq;
    no max-subtraction (logits ~N(0,1)); softmax denominator comes from a
    ones-column in the AV stationary; normalization deferred to the host
    (ships y_un + per-pixel denominators, host divides).

Speed structure. The scalar engine's exp stream is the pacer: 128 exp
instructions of [128,1024] at ~1.11us each (1 elem/cycle/partition at
1.2GHz + ~230ns instruction overhead) ~= 142us; everything else must hide
under it. Measured facts this schedule is built on (NTFF traces):
  * A 512-col matmul costs ~215ns streaming at the promoted 2.4GHz PE
    clock + ~100-135ns weight load; per attention iteration the PE runs
    QK(2) + AV(2) matmuls ~= 1.05-1.25us, just under the exp. fp8
    DoubleRow gives NO streaming speedup on this silicon (tried: 512-out
    DoubleRow measures ~375ns like bf16), so everything stays bf16.
  * The PE p-state: ~5us of continuous warmup matmuls promote 1.2->2.4GHz
    (~13us in, right before the first projection); the exp-paced stream's
    small per-iteration gaps then hold it.
  * Each early dma_start issued on the scalar ring costs ~3.5us of ACT
    sequencer time, so the scalar ring carries only late y-output DMAs;
    inputs ride the sync ring (priority slices first: first 512 xkv cols,
    first 1024 xq cols, weights at the head) and the gpsimd SWDGE queue.
  * The remaining k|v / q projections are woven into the chunk-0 stream at
    at most one matmul-pair per j (PE executes in order; heavier weaves
    starve the exp stream 1:1, lighter ones let the p-state drop).
  * v^T is produced per-1024-column piece: fused k|v projection, DVE
    drain, and four piece-granular hardware DMA transposes — the first AV
    only waits for piece 0, so only PRE=4 exps need banking and the
    end-of-stream AV debt (the kernel tail) stays small.
  * The AV stationary is trimmed to 65 columns (64 v^T + ones).
"""

import numpy as np
import ml_dtypes

import concourse.bacc as bacc
import concourse.mybir as mybir
import concourse.tile as tile
from concourse.bass_utils import run_bass_kernel_spmd

F32 = mybir.dt.float32
BF16 = mybir.dt.bfloat16

B, C, HGT, WID = 2, 256, 64, 64
S = HGT * WID  # 4096 pixels
NH, HD = 4, 64
NCORES = 8
P = 128
IC = 1024  # i-chunk width (2 PSUM banks)
NI = S // IC  # 4
NJ = S // P  # 32 j-blocks
SCALE = HD ** -0.5
KPRI = 512   # priority xkv columns (first k|v projection slice)
QPRI = 1024  # priority xq columns (chunk-0 q projection)
PRE = 4      # chunk-0 exps banked ahead of the first AV (covers the
             # first v-transpose piece)


def _emit(tc):
    nc = tc.nc
    xq = nc.dram_tensor("xq", [2, P, S], BF16, kind="ExternalInput").ap()
    xkv = nc.dram_tensor("xkv", [2, P, S], BF16, kind="ExternalInput").ap()
    wqT = nc.dram_tensor("wqT", [2, P, HD], BF16, kind="ExternalInput").ap()
    wkvT = nc.dram_tensor("wkvT", [2, P, P], BF16, kind="ExternalInput").ap()
    woT = nc.dram_tensor("woT", [HD, C], BF16, kind="ExternalInput").ap()
    bq = nc.dram_tensor("bq", [HD, 1], F32, kind="ExternalInput").ap()
    y = nc.dram_tensor("y", [2, P, S], F32, kind="ExternalOutput").ap()
    yden = nc.dram_tensor("yden", [1, S], F32, kind="ExternalOutput").ap()

    with (
        tc.tile_pool(name="const", bufs=1) as cpool,
        tc.tile_pool(name="xp", bufs=1) as xpool,
        tc.tile_pool(name="qkv", bufs=1) as qpool,
        tc.tile_pool(name="es", bufs=10) as epool,
        tc.tile_pool(name="epi", bufs=2) as fpool,
        tc.tile_pool(name="ps", bufs=2, space="PSUM") as pp,
    ):
        # ---- critical-path weights + priority slices head the sync ring --
        wkv_sb = cpool.tile([P, 2 * P], BF16)
        wq_sb = cpool.tile([P, 2 * HD], BF16)
        bq_sb = cpool.tile([HD, 1], F32)
        for cch in range(2):
            nc.sync.dma_start(wkv_sb[:, cch * P:(cch + 1) * P], wkvT[cch])
        for cch in range(2):
            nc.sync.dma_start(wq_sb[:, cch * HD:(cch + 1) * HD], wqT[cch])
        nc.sync.dma_start(bq_sb[:], bq)

        xq_sb = [xpool.tile([P, S], BF16, tag=f"xq{i}", name=f"xq_sb{i}")
                 for i in range(2)]
        xkv_sb = [xpool.tile([P, S], BF16, tag=f"xkv{i}", name=f"xkv_sb{i}")
                  for i in range(2)]
        # Inputs ride sync + SWDGE only: early dma_starts on the scalar
        # ring cost ~3.5us of ACT sequencer each, straight off the pacer.
        nc.sync.dma_start(xkv_sb[1][:, 0:KPRI], xkv[1][:, 0:KPRI])
        nc.sync.dma_start(xkv_sb[0][:, 0:KPRI], xkv[0][:, 0:KPRI])
        nc.sync.dma_start(xq_sb[1][:, 0:QPRI], xq[1][:, 0:QPRI])
        nc.sync.dma_start(xq_sb[0][:, 0:QPRI], xq[0][:, 0:QPRI])
        for s in range(1, 5):
            sl = slice(s * 512, (s + 1) * 512)
            nc.sync.dma_start(xkv_sb[0][:, sl], xkv[0][:, sl])

        # ---- the rest on the gpsimd SWDGE queue (self-issued) ----
        wo_sb = cpool.tile([HD, C], BF16)
        nc.gpsimd.dma_start(wo_sb[:], woT)
        for s in range(1, 5):
            sl = slice(s * 512, (s + 1) * 512)
            nc.gpsimd.dma_start(xkv_sb[1][:, sl], xkv[1][:, sl])
        for s in range(5, S // 512):
            sl = slice(s * 512, (s + 1) * 512)
            nc.gpsimd.dma_start(xkv_sb[1][:, sl], xkv[1][:, sl])
            nc.gpsimd.dma_start(xkv_sb[0][:, sl], xkv[0][:, sl])
        for t in range(1, S // 1024):
            sl = slice(t * 1024, (t + 1) * 1024)
            nc.gpsimd.dma_start(xq_sb[1][:, sl], xq[1][:, sl])

        # Zero bias for exp via memset (a float bias would become a DMA'd
        # const tensor queued behind the input DMAs).
        zbias_sb = cpool.tile([P, 1], F32)
        nc.vector.memset(zbias_sb[:], 0.0)

        # PE warmup burst: ~10us of dense matmuls while the input DMAs are
        # in flight; the activity monitor promotes the PE to 2.4GHz after
        # ~5us of sustained streaming, right before the first projection.
        wrm_sb = cpool.tile([P, 512], BF16)
        nc.vector.memset(wrm_sb[:], 0.0)
        for w in range(10):
            wp = pp.tile([P, 512], F32, tag="st", bufs=2, name="wp")
            nc.tensor.matmul(wp[:], wrm_sb[:, 0:P], wrm_sb[:],
                             start=True, stop=True)
        # Warmup exp so the ~2.7us activation-table load happens before the
        # first real exp.
        warm_sb = cpool.tile([P, 1], BF16)
        nc.scalar.activation(warm_sb[:], zbias_sb[:],
                             mybir.ActivationFunctionType.Exp,
                             bias=zbias_sb[:])

        # q/k zero-padded to 128 partitions (the zero rows contribute
        # nothing to the contraction).
        q_sb = qpool.tile([P, S], BF16)
        k_sb = qpool.tile([P, S], BF16)
        nc.vector.memset(q_sb[HD:P, :], 0.0)
        nc.vector.memset(k_sb[HD:P, :], 0.0)
        # v (dense, pre-transpose) lives on partitions 64:128 (the fused
        # k|v projection's PSUM rows), ready for the hardware transpose.
        v_sb = qpool.tile([P, S], BF16)
        # v^T blocks [j-part, (block, 128)]: cols 0:64 = v^T (the hardware
        # transpose requires the 128-stride block layout), col 64 = ones.
        # The AV stationary slices only cols 0:65.
        va_sb = qpool.tile([P, NJ * P], BF16)
        va_v = va_sb.rearrange("p (j c) -> p j c", c=P)
        nc.vector.memset(va_v[:, :, HD:HD + 1], 1.0)

        # ---- projections ----
        def kv_proj(s):
            # fused: stationary (wk^T | wv^T) -> PSUM rows 0:64 = k,
            # rows 64:128 = v, one matmul pass per 512-column slice
            sl = slice(s * 512, (s + 1) * 512)
            kvp = pp.tile([P, 512], F32, tag="st", bufs=2, name="kvp")
            nc.tensor.matmul(kvp[:], wkv_sb[:, 0:P], xkv_sb[0][:, sl],
                             start=True, stop=False)
            nc.tensor.matmul(kvp[:], wkv_sb[:, P:2 * P], xkv_sb[1][:, sl],
                             start=False, stop=True)
            nc.vector.tensor_copy(k_sb[0:HD, sl], kvp[0:HD, :])
            nc.vector.tensor_copy(v_sb[HD:P, sl], kvp[HD:P, :])

        def q_proj(t):
            sl = slice(t * 512, (t + 1) * 512)
            qp = pp.tile([HD, 512], F32, tag="st", bufs=2, name="qp")
            nc.tensor.matmul(qp[:], wq_sb[:, 0:HD], xq_sb[0][:, sl],
                             start=True, stop=False)
            nc.tensor.matmul(qp[:], wq_sb[:, HD:2 * HD], xq_sb[1][:, sl],
                             start=False, stop=True)
            nc.vector.tensor_scalar_add(q_sb[0:HD, sl], qp[:], bq_sb[:])

        kv_proj(0)
        q_proj(0)
        q_proj(1)

        def transpose_piece(g):
            # v^T for j-blocks 8g..8g+7, available as soon as v slices
            # 2g/2g+1 are drained — the first AV only needs piece 0.
            nc.sync.dma_start_transpose(
                out=va_v[:, 8 * g:8 * (g + 1), 0:HD],
                in_=v_sb[HD:P, 1024 * g:1024 * (g + 1)])

        def xq0_rest():
            for t in range(1, S // 1024):
                sl = slice(t * 1024, (t + 1) * 1024)
                nc.sync.dma_start(xq_sb[0][:, sl], xq[0][:, sl])

        # ---- attention ----
        def qk_exp(c, j):
            st = pp.tile([P, IC], F32, tag="st", bufs=2, name="st")
            for h in range(IC // 512):
                isl = slice(c * IC + h * 512, c * IC + (h + 1) * 512)
                nc.tensor.matmul(st[:, h * 512:(h + 1) * 512],
                                 k_sb[:, j * P:(j + 1) * P],
                                 q_sb[:, isl],
                                 start=True, stop=True)
            et = epool.tile([P, IC], BF16, name="et")
            nc.scalar.activation(et[:], st[:],
                                 mybir.ActivationFunctionType.Exp,
                                 bias=zbias_sb[:])
            return et

        pend = [None] * NI

        def epilogue_part2(i, final=False, ohs=(0, 1)):
            outt = pend[i]
            for oh in ohs:
                for h in range(IC // 512):
                    yp = pp.tile([P, 512], F32, tag="av", bufs=2, name="yp")
                    nc.tensor.matmul(yp[:], wo_sb[:, oh * P:(oh + 1) * P],
                                     outt[:, h * 512:(h + 1) * 512],
                                     start=True, stop=True)
                    ys = fpool.tile([P, 512], F32, name="ys")
                    if final and (oh + h) % 2 == 1:
                        nc.scalar.activation(
                            ys[:], yp[:], mybir.ActivationFunctionType.Copy)
                    else:
                        nc.vector.tensor_copy(ys[:], yp[:])
                    eng = nc.sync if oh == 0 else nc.scalar
                    eng.dma_start(
                        y[oh][:, i * IC + h * 512:i * IC + (h + 1) * 512],
                        ys[:])

        # Chunk-0 weave: remaining projections + transpose pieces ride the
        # exp-paced stream at at most one matmul-pair per j.
        weave0 = {
            0: [lambda: kv_proj(1)],
            1: [lambda: transpose_piece(0)],
            2: [lambda: kv_proj(2)],
            4: [lambda: kv_proj(3)],
            5: [lambda: transpose_piece(1)],
            6: [lambda: kv_proj(4)],
            8: [lambda: kv_proj(5)],
            9: [lambda: transpose_piece(2)],
            10: [lambda: kv_proj(6)],
            12: [lambda: kv_proj(7)],
            13: [lambda: transpose_piece(3)],
            14: [lambda: xq0_rest()],
            15: [lambda: q_proj(2)],
            17: [lambda: q_proj(3)],
            19: [lambda: q_proj(4)],
            21: [lambda: q_proj(5)],
            23: [lambda: q_proj(6)],
            25: [lambda: q_proj(7)],
        }

        bank = []
        for j in range(PRE):
            bank.append(qk_exp(0, j))
            for fn in weave0.get(j, []):
                fn()

        for i in range(NI):
            av = pp.tile([HD + 1, IC], F32, tag="av", bufs=2, name="av")
            for j in range(NJ):
                if i > 0 and j == 8:
                    epilogue_part2(i - 1, ohs=(0,))
                if i > 0 and j == 10:
                    epilogue_part2(i - 1, ohs=(1,))
                if i == 0 and j < PRE:
                    et = bank[j]
                else:
                    et = qk_exp(i, j)
                    if i == 0:
                        for fn in weave0.get(j, []):
                            fn()
                for h in range(IC // 512):
                    nc.tensor.matmul(av[:, h * 512:(h + 1) * 512],
                                     va_v[:, j, 0:HD + 1],
                                     et[:, h * 512:(h + 1) * 512],
                                     start=(j == 0), stop=(j == NJ - 1))

            outt = fpool.tile([HD, IC], BF16, name="outt")
            if i == NI - 1:
                nc.vector.tensor_copy(outt[:, 0:512], av[0:HD, 0:512])
                nc.vector.tensor_copy(outt[:, 512:IC], av[0:HD, 512:IC])
            else:
                nc.vector.tensor_copy(outt[:], av[0:HD, :])
            den = fpool.tile([1, IC], F32, name="den")
            nc.vector.tensor_copy(den[:], av[HD:HD + 1, :])
            nc.gpsimd.dma_start(yden[:, i * IC:(i + 1) * IC], den[:])
            pend[i] = outt

        epilogue_part2(NI - 1, final=True)


def build():
    nc = bacc.Bacc("TRN2", target_bir_lowering=False, debug=False,
                   enable_asserts=False)
    with tile.TileContext(nc) as tc:
        _emit(tc)
    nc.compile()
    return nc


_NC_CACHE = []


def _get_nc():
    if not _NC_CACHE:
        _NC_CACHE.append(build())
    return _NC_CACHE[0]


def make_in_maps(x_q, x_kv, wq, bq, wk, bk, wv, bv, wo, bo):
    bf = ml_dtypes.bfloat16
    in_maps = []
    bo_effs = []
    for c in range(NCORES):
        b, n = divmod(c, NH)
        hs = slice(n * HD, (n + 1) * HD)
        wq_h = wq[hs].astype(np.float64) * SCALE
        bo_eff = wo[:, hs].astype(np.float64) @ bv[hs].astype(np.float64)
        if n == 0:
            bo_eff = bo_eff + bo.astype(np.float64)
        bo_effs.append(bo_eff.astype(np.float32))
        in_maps.append({
            "xq": np.ascontiguousarray(
                x_q[b].reshape(C, S).reshape(2, P, S)).astype(bf),
            "xkv": np.ascontiguousarray(
                x_kv[b].reshape(C, S).reshape(2, P, S)).astype(bf),
            "wqT": np.ascontiguousarray(wq_h.T.reshape(2, P, HD)).astype(bf),
            "wkvT": np.ascontiguousarray(
                np.concatenate([wk[hs].T, wv[hs].T], axis=1)
                .reshape(2, P, P)).astype(bf),
            "woT": np.ascontiguousarray(wo[:, hs].T).astype(bf),
            "bq": (bq[hs].astype(np.float64) * SCALE
                   ).astype(np.float32).reshape(HD, 1),
        })
    return in_maps, bo_effs


def assemble_output(results, bo_effs):
    # y_core is the unnormalized head partial; divide by the softmax
    # denominator and add the (host-folded) bias here.
    y = np.zeros((B, C, S), np.float32)
    for c in range(NCORES):
        b = c // NH
        den = results[c]["yden"].reshape(1, S)
        y[b] += results[c]["y"].reshape(C, S) / den \
            + bo_effs[c].reshape(C, 1)
    return y.reshape(B, C, HGT, WID)


def kernel(**inputs):
    nc = _get_nc()
    in_maps, bo_effs = make_in_maps(**inputs)
    res = run_bass_kernel_spmd(nc, in_maps, list(range(NCORES)))
    return assemble_output(res.results, bo_effs)


if __name__ == "__main__":
    nc = build()
    print("built + compiled ok")


# revision 21
# speedup vs baseline: 1.0766x; 1.0011x over previous
"""Cross-attention kernel for Trainium2, sharded over 8 NeuronCores.

Problem (per reference):
  q = wq @ x_q + bq ; k = wk @ x_kv + bk ; v = wv @ x_kv + bv   (1x1 convs)
  per head: attn = softmax(q^T k / sqrt(hd)) ; out = attn @ v^T
  y = wo @ out + bo

Sharding: core c -> (batch b = c // 4, head n = c % 4). Each core runs one
head's full attention and produces the partial output projection
y_part = wo[:, head] @ out_head; the host sums the 4 head partials per batch.

Mathematically exact simplifications (as in the 192us version):
  * bk drops (per-query logit shift cancels in softmax); bv folds into the
    host-side bias (softmax rows sum to 1); scale folds into wq/bq;
    no max-subtraction (logits ~N(0,1)); softmax denominator comes from a
    ones-column in the AV stationary; normalization deferred to the host
    (ships y_un + per-pixel denominators, host divides).

Speed structure. The scalar engine's exp stream is the pacer: 128 exp
instructions of [128,1024] at ~1.11us each (1 elem/cycle/partition at
1.2GHz + ~230ns instruction overhead) ~= 142us; everything else must hide
under it. Measured facts this schedule is built on (NTFF traces):
  * A 512-col matmul costs ~215ns streaming at the promoted 2.4GHz PE
    clock + ~100-135ns weight load; per attention iteration the PE runs
    QK(2) + AV(2) matmuls ~= 1.05-1.25us, just under the exp. fp8
    DoubleRow gives NO streaming speedup on this silicon (tried: 512-out
    DoubleRow measures ~375ns like bf16), so everything stays bf16.
  * The PE p-state: ~5us of continuous warmup matmuls promote 1.2->2.4GHz
    (~13us in, right before the first projection); the exp-paced stream's
    small per-iteration gaps then hold it.
  * Each early dma_start issued on the scalar ring costs ~3.5us of ACT
    sequencer time, so the scalar ring carries only late y-output DMAs;
    inputs ride the sync ring (priority slices first: first 512 xkv cols,
    first 1024 xq cols, weights at the head) and the gpsimd SWDGE queue.
  * The remaining k|v / q projections are woven into the chunk-0 stream at
    at most one matmul-pair per j (PE executes in order; heavier weaves
    starve the exp stream 1:1, lighter ones let the p-state drop).
  * v^T is produced per-1024-column piece: fused k|v projection, DVE
    drain, and four piece-granular hardware DMA transposes — the first AV
    only waits for piece 0, so only PRE=4 exps need banking and the
    end-of-stream AV debt (the kernel tail) stays small.
  * The AV stationary is trimmed to 65 columns (64 v^T + ones).
"""

import numpy as np
import ml_dtypes

import concourse.bacc as bacc
import concourse.mybir as mybir
import concourse.tile as tile
from concourse.bass_utils import run_bass_kernel_spmd

F32 = mybir.dt.float32
BF16 = mybir.dt.bfloat16

B, C, HGT, WID = 2, 256, 64, 64
S = HGT * WID  # 4096 pixels
NH, HD = 4, 64
NCORES = 8
P = 128
IC = 1024  # i-chunk width (2 PSUM banks)
NI = S // IC  # 4
NJ = S // P  # 32 j-blocks
SCALE = HD ** -0.5
KPRI = 512   # priority xkv columns (first k|v projection slice)
QPRI = 1024  # priority xq columns (chunk-0 q projection)
PRE = 4      # chunk-0 exps banked ahead of the first AV (covers the
             # first v-transpose piece)


def _emit(tc):
    nc = tc.nc
    xq = nc.dram_tensor("xq", [2, P, S], BF16, kind="ExternalInput").ap()
    xkv = nc.dram_tensor("xkv", [2, P, S], BF16, kind="ExternalInput").ap()
    wqT = nc.dram_tensor("wqT", [2, P, HD], BF16, kind="ExternalInput").ap()
    wkvT = nc.dram_tensor("wkvT", [2, P, P], BF16, kind="ExternalInput").ap()
    woT = nc.dram_tensor("woT", [HD, C], BF16, kind="ExternalInput").ap()
    bq = nc.dram_tensor("bq", [HD, 1], F32, kind="ExternalInput").ap()
    y = nc.dram_tensor("y", [2, P, S], BF16, kind="ExternalOutput").ap()
    yden = nc.dram_tensor("yden", [1, S], F32, kind="ExternalOutput").ap()

    with (
        tc.tile_pool(name="const", bufs=1) as cpool,
        tc.tile_pool(name="xp", bufs=1) as xpool,
        tc.tile_pool(name="qkv", bufs=1) as qpool,
        tc.tile_pool(name="es", bufs=10) as epool,
        tc.tile_pool(name="epi", bufs=2) as fpool,
        tc.tile_pool(name="ps", bufs=2, space="PSUM") as pp,
    ):
        # ---- critical-path weights + priority slices head the sync ring --
        wkv_sb = cpool.tile([P, 2 * P], BF16)
        wq_sb = cpool.tile([P, 2 * HD], BF16)
        bq_sb = cpool.tile([HD, 1], F32)
        for cch in range(2):
            nc.sync.dma_start(wkv_sb[:, cch * P:(cch + 1) * P], wkvT[cch])
        for cch in range(2):
            nc.sync.dma_start(wq_sb[:, cch * HD:(cch + 1) * HD], wqT[cch])
        nc.sync.dma_start(bq_sb[:], bq)

        xq_sb = [xpool.tile([P, S], BF16, tag=f"xq{i}", name=f"xq_sb{i}")
                 for i in range(2)]
        xkv_sb = [xpool.tile([P, S], BF16, tag=f"xkv{i}", name=f"xkv_sb{i}")
                  for i in range(2)]
        # Inputs ride sync + SWDGE only: early dma_starts on the scalar
        # ring cost ~3.5us of ACT sequencer each, straight off the pacer.
        nc.sync.dma_start(xkv_sb[1][:, 0:KPRI], xkv[1][:, 0:KPRI])
        nc.sync.dma_start(xkv_sb[0][:, 0:KPRI], xkv[0][:, 0:KPRI])
        nc.sync.dma_start(xq_sb[0][:, 0:QPRI], xq[0][:, 0:QPRI])
        for s in range(1, 5):
            sl = slice(s * 512, (s + 1) * 512)
            nc.sync.dma_start(xkv_sb[0][:, sl], xkv[0][:, sl])

        # ---- the rest on the gpsimd SWDGE queue (self-issued; starts
        # ~8us in — the second priority xq half rides at its head so the
        # two priority queues fill in parallel) ----
        nc.gpsimd.dma_start(xq_sb[1][:, 0:QPRI], xq[1][:, 0:QPRI])
        wo_sb = cpool.tile([HD, C], BF16)
        nc.gpsimd.dma_start(wo_sb[:], woT)
        for s in range(1, 5):
            sl = slice(s * 512, (s + 1) * 512)
            nc.gpsimd.dma_start(xkv_sb[1][:, sl], xkv[1][:, sl])
        for s in range(5, S // 512):
            sl = slice(s * 512, (s + 1) * 512)
            nc.gpsimd.dma_start(xkv_sb[1][:, sl], xkv[1][:, sl])
            nc.gpsimd.dma_start(xkv_sb[0][:, sl], xkv[0][:, sl])
        for t in range(1, S // 1024):
            sl = slice(t * 1024, (t + 1) * 1024)
            nc.gpsimd.dma_start(xq_sb[1][:, sl], xq[1][:, sl])

        # Zero bias for exp via memset (a float bias would become a DMA'd
        # const tensor queued behind the input DMAs).
        zbias_sb = cpool.tile([P, 1], F32)
        nc.vector.memset(zbias_sb[:], 0.0)

        # PE warmup burst: ~10us of dense matmuls while the input DMAs are
        # in flight; the activity monitor promotes the PE to 2.4GHz after
        # ~5us of sustained streaming, right before the first projection.
        wrm_sb = cpool.tile([P, 512], BF16)
        nc.vector.memset(wrm_sb[:], 0.0)
        for w in range(10):
            wp = pp.tile([P, 512], F32, tag="st", bufs=2, name="wp")
            nc.tensor.matmul(wp[:], wrm_sb[:, 0:P], wrm_sb[:],
                             start=True, stop=True)
        # Warmup exp so the ~2.7us activation-table load happens before the
        # first real exp.
        warm_sb = cpool.tile([P, 1], BF16)
        nc.scalar.activation(warm_sb[:], zbias_sb[:],
                             mybir.ActivationFunctionType.Exp,
                             bias=zbias_sb[:])

        # q/k zero-padded to 128 partitions (the zero rows contribute
        # nothing to the contraction).
        q_sb = qpool.tile([P, S], BF16)
        k_sb = qpool.tile([P, S], BF16)
        nc.vector.memset(q_sb[HD:P, :], 0.0)
        nc.vector.memset(k_sb[HD:P, :], 0.0)
        # v (dense, pre-transpose) lives on partitions 64:128 (the fused
        # k|v projection's PSUM rows), ready for the hardware transpose.
        v_sb = qpool.tile([P, S], BF16)
        # v^T blocks [j-part, (block, 128)]: cols 0:64 = v^T (the hardware
        # transpose requires the 128-stride block layout), col 64 = ones.
        # The AV stationary slices only cols 0:65.
        va_sb = qpool.tile([P, NJ * P], BF16)
        va_v = va_sb.rearrange("p (j c) -> p j c", c=P)
        nc.vector.memset(va_v[:, :, HD:HD + 1], 1.0)

        # ---- projections ----
        def kv_proj(s):
            # fused: stationary (wk^T | wv^T) -> PSUM rows 0:64 = k,
            # rows 64:128 = v, one matmul pass per 512-column slice
            sl = slice(s * 512, (s + 1) * 512)
            kvp = pp.tile([P, 512], F32, tag="st", bufs=2, name="kvp")
            nc.tensor.matmul(kvp[:], wkv_sb[:, 0:P], xkv_sb[0][:, sl],
                             start=True, stop=False)
            nc.tensor.matmul(kvp[:], wkv_sb[:, P:2 * P], xkv_sb[1][:, sl],
                             start=False, stop=True)
            nc.vector.tensor_copy(k_sb[0:HD, sl], kvp[0:HD, :])
            nc.vector.tensor_copy(v_sb[HD:P, sl], kvp[HD:P, :])

        def q_proj(t):
            sl = slice(t * 512, (t + 1) * 512)
            qp = pp.tile([HD, 512], F32, tag="st", bufs=2, name="qp")
            nc.tensor.matmul(qp[:], wq_sb[:, 0:HD], xq_sb[0][:, sl],
                             start=True, stop=False)
            nc.tensor.matmul(qp[:], wq_sb[:, HD:2 * HD], xq_sb[1][:, sl],
                             start=False, stop=True)
            nc.vector.tensor_scalar_add(q_sb[0:HD, sl], qp[:], bq_sb[:])

        kv_proj(0)
        q_proj(0)
        q_proj(1)

        def transpose_piece(g):
            # v^T for j-blocks 8g..8g+7, available as soon as v slices
            # 2g/2g+1 are drained — the first AV only needs piece 0.
            nc.sync.dma_start_transpose(
                out=va_v[:, 8 * g:8 * (g + 1), 0:HD],
                in_=v_sb[HD:P, 1024 * g:1024 * (g + 1)])

        def xq0_rest():
            for t in range(1, S // 1024):
                sl = slice(t * 1024, (t + 1) * 1024)
                nc.sync.dma_start(xq_sb[0][:, sl], xq[0][:, sl])

        # ---- attention ----
        def qk_exp(c, j):
            st = pp.tile([P, IC], F32, tag="st", bufs=2, name="st")
            for h in range(IC // 512):
                isl = slice(c * IC + h * 512, c * IC + (h + 1) * 512)
                nc.tensor.matmul(st[:, h * 512:(h + 1) * 512],
                                 k_sb[:, j * P:(j + 1) * P],
                                 q_sb[:, isl],
                                 start=True, stop=True)
            et = epool.tile([P, IC], BF16, name="et")
            nc.scalar.activation(et[:], st[:],
                                 mybir.ActivationFunctionType.Exp,
                                 bias=zbias_sb[:])
            return et

        pend = [None] * NI

        def epilogue_part2(i, final=False, ohs=(0, 1)):
            outt = pend[i]
            for oh in ohs:
                for h in range(IC // 512):
                    yp = pp.tile([P, 512], F32, tag="av", bufs=2, name="yp")
                    nc.tensor.matmul(yp[:], wo_sb[:, oh * P:(oh + 1) * P],
                                     outt[:, h * 512:(h + 1) * 512],
                                     start=True, stop=True)
                    ys = fpool.tile([P, 512], BF16, name="ys")
                    if final and (oh + h) % 2 == 1:
                        nc.scalar.activation(
                            ys[:], yp[:], mybir.ActivationFunctionType.Copy)
                    else:
                        nc.vector.tensor_copy(ys[:], yp[:])
                    eng = nc.sync if oh == 0 else nc.scalar
                    eng.dma_start(
                        y[oh][:, i * IC + h * 512:i * IC + (h + 1) * 512],
                        ys[:])

        # Chunk-0 weave: remaining projections + transpose pieces ride the
        # exp-paced stream at at most one matmul-pair per j.
        weave0 = {
            0: [lambda: kv_proj(1)],
            1: [lambda: transpose_piece(0)],
            2: [lambda: kv_proj(2)],
            4: [lambda: kv_proj(3)],
            5: [lambda: transpose_piece(1)],
            6: [lambda: kv_proj(4)],
            8: [lambda: kv_proj(5)],
            9: [lambda: transpose_piece(2)],
            10: [lambda: kv_proj(6)],
            12: [lambda: kv_proj(7)],
            13: [lambda: transpose_piece(3)],
            14: [lambda: xq0_rest()],
            15: [lambda: q_proj(2)],
            17: [lambda: q_proj(3)],
            19: [lambda: q_proj(4)],
            21: [lambda: q_proj(5)],
            23: [lambda: q_proj(6)],
            25: [lambda: q_proj(7)],
        }

        bank = []
        for j in range(PRE):
            bank.append(qk_exp(0, j))
            for fn in weave0.get(j, []):
                fn()

        for i in range(NI):
            av = pp.tile([HD + 1, IC], F32, tag="av", bufs=2, name="av")
            for j in range(NJ):
                if i > 0 and j == 8:
                    epilogue_part2(i - 1, ohs=(0,))
                if i > 0 and j == 10:
                    epilogue_part2(i - 1, ohs=(1,))
                if i == 0 and j < PRE:
                    et = bank[j]
                else:
                    et = qk_exp(i, j)
                    if i == 0:
                        for fn in weave0.get(j, []):
                            fn()
                for h in (1, 0):
                    nc.tensor.matmul(av[:, h * 512:(h + 1) * 512],
                                     va_v[:, j, 0:HD + 1],
                                     et[:, h * 512:(h + 1) * 512],
                                     start=(j == 0), stop=(j == NJ - 1))

            outt = fpool.tile([HD, IC], BF16, name="outt")
            if i == NI - 1:
                nc.vector.tensor_copy(outt[:, 0:512], av[0:HD, 0:512])
                nc.vector.tensor_copy(outt[:, 512:IC], av[0:HD, 512:IC])
            else:
                nc.vector.tensor_copy(outt[:], av[0:HD, :])
            den = fpool.tile([1, IC], F32, name="den")
            nc.vector.tensor_copy(den[:], av[HD:HD + 1, :])
            nc.gpsimd.dma_start(yden[:, i * IC:(i + 1) * IC], den[:])
            pend[i] = outt

        epilogue_part2(NI - 1, final=True)


def build():
    nc = bacc.Bacc("TRN2", target_bir_lowering=False, debug=False,
                   enable_asserts=False)
    with tile.TileContext(nc) as tc:
        _emit(tc)
    nc.compile()
    return nc


_NC_CACHE = []


def _get_nc():
    if not _NC_CACHE:
        _NC_CACHE.append(build())
    return _NC_CACHE[0]


def make_in_maps(x_q, x_kv, wq, bq, wk, bk, wv, bv, wo, bo):
    bf = ml_dtypes.bfloat16
    in_maps = []
    bo_effs = []
    for c in range(NCORES):
        b, n = divmod(c, NH)
        hs = slice(n * HD, (n + 1) * HD)
        wq_h = wq[hs].astype(np.float64) * SCALE
        bo_eff = wo[:, hs].astype(np.float64) @ bv[hs].astype(np.float64)
        if n == 0:
            bo_eff = bo_eff + bo.astype(np.float64)
        bo_effs.append(bo_eff.astype(np.float32))
        in_maps.append({
            "xq": np.ascontiguousarray(
                x_q[b].reshape(C, S).reshape(2, P, S)).astype(bf),
            "xkv": np.ascontiguousarray(
                x_kv[b].reshape(C, S).reshape(2, P, S)).astype(bf),
            "wqT": np.ascontiguousarray(wq_h.T.reshape(2, P, HD)).astype(bf),
            "wkvT": np.ascontiguousarray(
                np.concatenate([wk[hs].T, wv[hs].T], axis=1)
                .reshape(2, P, P)).astype(bf),
            "woT": np.ascontiguousarray(wo[:, hs].T).astype(bf),
            "bq": (bq[hs].astype(np.float64) * SCALE
                   ).astype(np.float32).reshape(HD, 1),
        })
    return in_maps, bo_effs


def assemble_output(results, bo_effs):
    # y_core is the unnormalized head partial; divide by the softmax
    # denominator and add the (host-folded) bias here.
    y = np.zeros((B, C, S), np.float32)
    for c in range(NCORES):
        b = c // NH
        den = results[c]["yden"].reshape(1, S)
        y[b] += results[c]["y"].astype(np.float32).reshape(C, S) / den \
            + bo_effs[c].reshape(C, 1)
    return y.reshape(B, C, HGT, WID)


def kernel(**inputs):
    nc = _get_nc()
    in_maps, bo_effs = make_in_maps(**inputs)
    res = run_bass_kernel_spmd(nc, in_maps, list(range(NCORES)))
    return assemble_output(res.results, bo_effs)


if __name__ == "__main__":
    nc = build()
    print("built + compiled ok")


# revision 24
# speedup vs baseline: 1.1306x; 1.0501x over previous
"""Cross-attention kernel for Trainium2, sharded over 8 NeuronCores.

Problem (per reference):
  q = wq @ x_q + bq ; k = wk @ x_kv + bk ; v = wv @ x_kv + bv   (1x1 convs)
  per head: attn = softmax(q^T k / sqrt(hd)) ; out = attn @ v^T
  y = wo @ out + bo

Sharding: core c -> (batch b = c // 4, head n = c % 4). Each core runs one
head's full attention and produces the partial output projection
y_part = wo[:, head] @ out_head; the host sums the 4 head partials per batch.

Mathematically exact simplifications (as in the 192us version):
  * bk drops (per-query logit shift cancels in softmax); bv folds into the
    host-side bias (softmax rows sum to 1); scale folds into wq/bq;
    no max-subtraction (logits ~N(0,1)); softmax denominator comes from a
    ones-column in the AV stationary; normalization deferred to the host
    (ships y_un + per-pixel denominators, host divides).

Speed structure. The scalar engine's exp stream is the pacer: 128 exp
instructions of [128,1024] at ~1.11us each (1 elem/cycle/partition at
1.2GHz + ~230ns instruction overhead) ~= 142us; everything else must hide
under it. Measured facts this schedule is built on (NTFF traces):
  * A 512-col matmul costs ~215ns streaming at the promoted 2.4GHz PE
    clock + ~100-135ns weight load; per attention iteration the PE runs
    QK(2) + AV(2) matmuls ~= 1.05-1.25us, just under the exp. fp8
    DoubleRow gives NO streaming speedup on this silicon (tried: 512-out
    DoubleRow measures ~375ns like bf16), so everything stays bf16.
  * The PE p-state: ~5us of continuous warmup matmuls promote 1.2->2.4GHz
    (~13us in, right before the first projection); the exp-paced stream's
    small per-iteration gaps then hold it.
  * Each early dma_start issued on the scalar ring costs ~3.5us of ACT
    sequencer time, so the scalar ring carries only late y-output DMAs;
    inputs ride the sync ring (priority slices first: first 512 xkv cols,
    first 1024 xq cols, weights at the head) and the gpsimd SWDGE queue.
  * The remaining k|v / q projections are woven into the chunk-0 stream at
    at most one matmul-pair per j (PE executes in order; heavier weaves
    starve the exp stream 1:1, lighter ones let the p-state drop).
  * v^T is produced per-1024-column piece: fused k|v projection, DVE
    drain, and four piece-granular hardware DMA transposes — the first AV
    only waits for piece 0, so only PRE=4 exps need banking and the
    end-of-stream AV debt (the kernel tail) stays small.
  * The AV stationary is trimmed to 65 columns (64 v^T + ones).
"""

import numpy as np
import ml_dtypes

import concourse.bacc as bacc
import concourse.mybir as mybir
import concourse.tile as tile
from concourse.bass_utils import run_bass_kernel_spmd

F32 = mybir.dt.float32
BF16 = mybir.dt.bfloat16

B, C, HGT, WID = 2, 256, 64, 64
S = HGT * WID  # 4096 pixels
NH, HD = 4, 64
NCORES = 8
P = 128
IC = 1024  # i-chunk width (2 PSUM banks)
NI = S // IC  # 4
NJ = S // P  # 32 j-blocks
SCALE = HD ** -0.5
KPRI = 512   # priority xkv columns (first k|v projection slice)
QPRI = 1024  # priority xq columns (chunk-0 q projection)
PRE = 6      # chunk-0 exps banked ahead of the first AV (covers the
             # first v-transpose piece)


def _emit(tc):
    nc = tc.nc
    xq = nc.dram_tensor("xq", [2, P, S], BF16, kind="ExternalInput").ap()
    xkv = nc.dram_tensor("xkv", [2, P, S], BF16, kind="ExternalInput").ap()
    wqT = nc.dram_tensor("wqT", [2, P, HD], BF16, kind="ExternalInput").ap()
    wkvT = nc.dram_tensor("wkvT", [2, P, P], BF16, kind="ExternalInput").ap()
    woT = nc.dram_tensor("woT", [HD, C], BF16, kind="ExternalInput").ap()
    bq = nc.dram_tensor("bq", [HD, 1], F32, kind="ExternalInput").ap()
    y = nc.dram_tensor("y", [2, P, S], BF16, kind="ExternalOutput").ap()
    yden = nc.dram_tensor("yden", [1, S], F32, kind="ExternalOutput").ap()

    with (
        tc.tile_pool(name="const", bufs=1) as cpool,
        tc.tile_pool(name="xp", bufs=1) as xpool,
        tc.tile_pool(name="qkv", bufs=1) as qpool,
        tc.tile_pool(name="es", bufs=10) as epool,
        tc.tile_pool(name="epi", bufs=2) as fpool,
        tc.tile_pool(name="ps", bufs=2, space="PSUM") as pp,
    ):
        # ---- critical-path weights + priority slices head the sync ring --
        wkv_sb = cpool.tile([P, 2 * P], BF16)
        wq_sb = cpool.tile([P, 2 * HD], BF16)
        bq_sb = cpool.tile([HD, 1], F32)
        for cch in range(2):
            nc.sync.dma_start(wkv_sb[:, cch * P:(cch + 1) * P], wkvT[cch])
        for cch in range(2):
            nc.sync.dma_start(wq_sb[:, cch * HD:(cch + 1) * HD], wqT[cch])
        nc.sync.dma_start(bq_sb[:], bq)

        xq_sb = [xpool.tile([P, S], BF16, tag=f"xq{i}", name=f"xq_sb{i}")
                 for i in range(2)]
        xkv_sb = [xpool.tile([P, S], BF16, tag=f"xkv{i}", name=f"xkv_sb{i}")
                  for i in range(2)]
        # Inputs ride sync + SWDGE only: early dma_starts on the scalar
        # ring cost ~3.5us of ACT sequencer each, straight off the pacer.
        nc.sync.dma_start(xkv_sb[1][:, 0:KPRI], xkv[1][:, 0:KPRI])
        nc.sync.dma_start(xkv_sb[0][:, 0:KPRI], xkv[0][:, 0:KPRI])
        nc.sync.dma_start(xq_sb[0][:, 0:QPRI], xq[0][:, 0:QPRI])
        for s in range(1, 5):
            sl = slice(s * 512, (s + 1) * 512)
            nc.sync.dma_start(xkv_sb[0][:, sl], xkv[0][:, sl])

        # ---- the rest on the gpsimd SWDGE queue (self-issued; starts
        # ~8us in — the second priority xq half rides at its head so the
        # two priority queues fill in parallel) ----
        nc.gpsimd.dma_start(xq_sb[1][:, 0:QPRI], xq[1][:, 0:QPRI])
        wo_sb = cpool.tile([HD, C], BF16)
        nc.gpsimd.dma_start(wo_sb[:], woT)
        for s in range(1, 5):
            sl = slice(s * 512, (s + 1) * 512)
            nc.gpsimd.dma_start(xkv_sb[1][:, sl], xkv[1][:, sl])
        for s in range(5, S // 512):
            sl = slice(s * 512, (s + 1) * 512)
            nc.gpsimd.dma_start(xkv_sb[1][:, sl], xkv[1][:, sl])
            nc.gpsimd.dma_start(xkv_sb[0][:, sl], xkv[0][:, sl])
        for t in range(1, S // 1024):
            sl = slice(t * 1024, (t + 1) * 1024)
            nc.gpsimd.dma_start(xq_sb[1][:, sl], xq[1][:, sl])

        # Zero bias for exp via memset (a float bias would become a DMA'd
        # const tensor queued behind the input DMAs).
        zbias_sb = cpool.tile([P, 1], F32)
        nc.vector.memset(zbias_sb[:], 0.0)

        # PE warmup burst: ~10us of dense matmuls while the input DMAs are
        # in flight; the activity monitor promotes the PE to 2.4GHz after
        # ~5us of sustained streaming, right before the first projection.
        wrm_sb = cpool.tile([P, 512], BF16)
        nc.vector.memset(wrm_sb[:], 0.0)
        for w in range(10):
            wp = pp.tile([P, 512], F32, tag="st", bufs=2, name="wp")
            nc.tensor.matmul(wp[:], wrm_sb[:, 0:P], wrm_sb[:],
                             start=True, stop=True)
        # Warmup exp so the ~2.7us activation-table load happens before the
        # first real exp.
        warm_sb = cpool.tile([P, 1], BF16)
        nc.scalar.activation(warm_sb[:], zbias_sb[:],
                             mybir.ActivationFunctionType.Exp,
                             bias=zbias_sb[:])

        # q/k zero-padded to 128 partitions (the zero rows contribute
        # nothing to the contraction). Only the slices the first exps need
        # are zeroed up front: a full-width [64,4096] DVE memset costs
        # ~3.5us and would queue ahead of the projection drains, gating the
        # first exp; the rest is deferred until the exp stream is running.
        q_sb = qpool.tile([P, S], BF16)
        k_sb = qpool.tile([P, S], BF16)
        nc.vector.memset(k_sb[HD:P, 0:KPRI], 0.0)
        nc.vector.memset(q_sb[HD:P, 0:QPRI], 0.0)
        # v (dense, pre-transpose) lives on partitions 64:128 (the fused
        # k|v projection's PSUM rows), ready for the hardware transpose.
        v_sb = qpool.tile([P, S], BF16)
        # v^T blocks [j-part, (block, 128)]: cols 0:64 = v^T (the hardware
        # transpose requires the 128-stride block layout), col 64 = ones.
        # The AV stationary slices only cols 0:65.
        va_sb = qpool.tile([P, NJ * P], BF16)
        va_v = va_sb.rearrange("p (j c) -> p j c", c=P)
        nc.vector.memset(va_v[:, :, HD:HD + 1], 1.0)

        # ---- projections ----
        def kv_proj(s):
            # fused: stationary (wk^T | wv^T) -> PSUM rows 0:64 = k,
            # rows 64:128 = v, one matmul pass per 512-column slice
            sl = slice(s * 512, (s + 1) * 512)
            kvp = pp.tile([P, 512], F32, tag="av", bufs=2, name="kvp")
            nc.tensor.matmul(kvp[:], wkv_sb[:, 0:P], xkv_sb[0][:, sl],
                             start=True, stop=False)
            nc.tensor.matmul(kvp[:], wkv_sb[:, P:2 * P], xkv_sb[1][:, sl],
                             start=False, stop=True)
            nc.vector.tensor_copy(k_sb[0:HD, sl], kvp[0:HD, :])
            nc.vector.tensor_copy(v_sb[HD:P, sl], kvp[HD:P, :])

        def q_proj(t):
            sl = slice(t * 512, (t + 1) * 512)
            qp = pp.tile([HD, 512], F32, tag="av", bufs=2, name="qp")
            nc.tensor.matmul(qp[:], wq_sb[:, 0:HD], xq_sb[0][:, sl],
                             start=True, stop=False)
            nc.tensor.matmul(qp[:], wq_sb[:, HD:2 * HD], xq_sb[1][:, sl],
                             start=False, stop=True)
            nc.vector.tensor_scalar_add(q_sb[0:HD, sl], qp[:], bq_sb[:])

        kv_proj(0)
        q_proj(0)
        q_proj(1)
        # deferred zero-pad remainders: DVE runs these ~16-22us while the
        # exp stream is already going; k columns 512:640 (j-block 4) are
        # needed first, at ~21us
        nc.vector.memset(k_sb[HD:P, KPRI:S], 0.0)
        nc.vector.memset(q_sb[HD:P, QPRI:S], 0.0)

        def transpose_piece(g):
            # v^T for j-blocks 8g..8g+7, available as soon as v slices
            # 2g/2g+1 are drained — the first AV only needs piece 0.
            nc.sync.dma_start_transpose(
                out=va_v[:, 8 * g:8 * (g + 1), 0:HD],
                in_=v_sb[HD:P, 1024 * g:1024 * (g + 1)])

        def xq0_rest():
            for t in range(1, S // 1024):
                sl = slice(t * 1024, (t + 1) * 1024)
                nc.sync.dma_start(xq_sb[0][:, sl], xq[0][:, sl])

        # ---- attention ----
        def qk_exp(c, j):
            st = pp.tile([P, IC], F32, tag="st", bufs=2, name="st")
            for h in range(IC // 512):
                isl = slice(c * IC + h * 512, c * IC + (h + 1) * 512)
                nc.tensor.matmul(st[:, h * 512:(h + 1) * 512],
                                 k_sb[:, j * P:(j + 1) * P],
                                 q_sb[:, isl],
                                 start=True, stop=True)
            et = epool.tile([P, IC], BF16, name="et")
            nc.scalar.activation(et[:], st[:],
                                 mybir.ActivationFunctionType.Exp,
                                 bias=zbias_sb[:])
            return et

        pend = [None] * NI

        def epilogue_part2(i, final=False, ohs=(0, 1)):
            outt = pend[i]
            for oh in ohs:
                for h in range(IC // 512):
                    yp = pp.tile([P, 512], F32, tag="av", bufs=2, name="yp")
                    nc.tensor.matmul(yp[:], wo_sb[:, oh * P:(oh + 1) * P],
                                     outt[:, h * 512:(h + 1) * 512],
                                     start=True, stop=True)
                    ys = fpool.tile([P, 512], BF16, name="ys")
                    if final and (oh + h) % 2 == 1:
                        nc.scalar.activation(
                            ys[:], yp[:], mybir.ActivationFunctionType.Copy)
                    else:
                        nc.vector.tensor_copy(ys[:], yp[:])
                    eng = nc.sync if oh == 0 else nc.scalar
                    eng.dma_start(
                        y[oh][:, i * IC + h * 512:i * IC + (h + 1) * 512],
                        ys[:])

        # Chunk-0 weave: remaining projections + transpose pieces ride the
        # exp-paced stream at at most one matmul-pair per j.
        weave0 = {
            0: [lambda: kv_proj(1)],
            1: [lambda: transpose_piece(0)],
            2: [lambda: kv_proj(2)],
            4: [lambda: kv_proj(3)],
            5: [lambda: transpose_piece(1)],
            6: [lambda: kv_proj(4)],
            8: [lambda: kv_proj(5)],
            9: [lambda: transpose_piece(2)],
            10: [lambda: kv_proj(6)],
            12: [lambda: kv_proj(7)],
            13: [lambda: transpose_piece(3)],
            14: [lambda: xq0_rest()],
            15: [lambda: q_proj(2)],
            17: [lambda: q_proj(3)],
            19: [lambda: q_proj(4)],
            21: [lambda: q_proj(5)],
            23: [lambda: q_proj(6)],
            25: [lambda: q_proj(7)],
        }

        bank = []
        for j in range(PRE):
            bank.append(qk_exp(0, j))
            for fn in weave0.get(j, []):
                fn()

        for i in range(NI):
            av = pp.tile([HD + 1, IC], F32, tag="av", bufs=2, name="av")
            for j in range(NJ):
                if i > 0 and j == 8:
                    epilogue_part2(i - 1, ohs=(0,))
                if i > 0 and j == 10:
                    epilogue_part2(i - 1, ohs=(1,))
                if i == 0 and j < PRE:
                    et = bank[j]
                else:
                    et = qk_exp(i, j)
                    if i == 0:
                        for fn in weave0.get(j, []):
                            fn()
                for h in (1, 0):
                    nc.tensor.matmul(av[:, h * 512:(h + 1) * 512],
                                     va_v[:, j, 0:HD + 1],
                                     et[:, h * 512:(h + 1) * 512],
                                     start=(j == 0), stop=(j == NJ - 1))

            outt = fpool.tile([HD, IC], BF16, name="outt")
            if i == NI - 1:
                nc.vector.tensor_copy(outt[:, 0:512], av[0:HD, 0:512])
                nc.vector.tensor_copy(outt[:, 512:IC], av[0:HD, 512:IC])
            else:
                nc.vector.tensor_copy(outt[:], av[0:HD, :])
            den = fpool.tile([1, IC], F32, name="den")
            nc.vector.tensor_copy(den[:], av[HD:HD + 1, :])
            nc.gpsimd.dma_start(yden[:, i * IC:(i + 1) * IC], den[:])
            pend[i] = outt

        epilogue_part2(NI - 1, final=True)


def build():
    nc = bacc.Bacc("TRN2", target_bir_lowering=False, debug=False,
                   enable_asserts=False)
    with tile.TileContext(nc) as tc:
        _emit(tc)
    nc.compile()
    return nc


_NC_CACHE = []


def _get_nc():
    if not _NC_CACHE:
        _NC_CACHE.append(build())
    return _NC_CACHE[0]


def make_in_maps(x_q, x_kv, wq, bq, wk, bk, wv, bv, wo, bo):
    bf = ml_dtypes.bfloat16
    in_maps = []
    bo_effs = []
    for c in range(NCORES):
        b, n = divmod(c, NH)
        hs = slice(n * HD, (n + 1) * HD)
        wq_h = wq[hs].astype(np.float64) * SCALE
        bo_eff = wo[:, hs].astype(np.float64) @ bv[hs].astype(np.float64)
        if n == 0:
            bo_eff = bo_eff + bo.astype(np.float64)
        bo_effs.append(bo_eff.astype(np.float32))
        in_maps.append({
            "xq": np.ascontiguousarray(
                x_q[b].reshape(C, S).reshape(2, P, S)).astype(bf),
            "xkv": np.ascontiguousarray(
                x_kv[b].reshape(C, S).reshape(2, P, S)).astype(bf),
            "wqT": np.ascontiguousarray(wq_h.T.reshape(2, P, HD)).astype(bf),
            "wkvT": np.ascontiguousarray(
                np.concatenate([wk[hs].T, wv[hs].T], axis=1)
                .reshape(2, P, P)).astype(bf),
            "woT": np.ascontiguousarray(wo[:, hs].T).astype(bf),
            "bq": (bq[hs].astype(np.float64) * SCALE
                   ).astype(np.float32).reshape(HD, 1),
        })
    return in_maps, bo_effs


def assemble_output(results, bo_effs):
    # y_core is the unnormalized head partial; divide by the softmax
    # denominator and add the (host-folded) bias here.
    y = np.zeros((B, C, S), np.float32)
    for c in range(NCORES):
        b = c // NH
        den = results[c]["yden"].reshape(1, S)
        y[b] += results[c]["y"].astype(np.float32).reshape(C, S) / den \
            + bo_effs[c].reshape(C, 1)
    return y.reshape(B, C, HGT, WID)


def kernel(**inputs):
    nc = _get_nc()
    in_maps, bo_effs = make_in_maps(**inputs)
    res = run_bass_kernel_spmd(nc, in_maps, list(range(NCORES)))
    return assemble_output(res.results, bo_effs)


if __name__ == "__main__":
    nc = build()
    print("built + compiled ok")


# revision 26
# speedup vs baseline: 1.1431x; 1.0111x over previous
"""Cross-attention kernel for Trainium2, sharded over 8 NeuronCores.

Problem (per reference):
  q = wq @ x_q + bq ; k = wk @ x_kv + bk ; v = wv @ x_kv + bv   (1x1 convs)
  per head: attn = softmax(q^T k / sqrt(hd)) ; out = attn @ v^T
  y = wo @ out + bo

Sharding: core c -> (batch b = c // 4, head n = c % 4). Each core runs one
head's full attention and produces the partial output projection
y_part = wo[:, head] @ out_head; the host sums the 4 head partials per batch.

Mathematically exact simplifications (as in the 192us version):
  * bk drops (per-query logit shift cancels in softmax); bv folds into the
    host-side bias (softmax rows sum to 1); scale folds into wq/bq;
    no max-subtraction (logits ~N(0,1)); softmax denominator comes from a
    ones-column in the AV stationary; normalization deferred to the host
    (ships y_un + per-pixel denominators, host divides).

Speed structure. The scalar engine's exp stream is the pacer: 128 exp
instructions of [128,1024] at ~1.11us each (1 elem/cycle/partition at
1.2GHz + ~230ns instruction overhead) ~= 142us; everything else must hide
under it. Measured facts this schedule is built on (NTFF traces):
  * A 512-col matmul costs ~215ns streaming at the promoted 2.4GHz PE
    clock + ~100-135ns weight load; per attention iteration the PE runs
    QK(2) + AV(2) matmuls ~= 1.05-1.25us, just under the exp. fp8
    DoubleRow gives NO streaming speedup on this silicon (tried: 512-out
    DoubleRow measures ~375ns like bf16), so everything stays bf16.
  * The PE p-state: ~5us of continuous warmup matmuls promote 1.2->2.4GHz
    (~13us in, right before the first projection); the exp-paced stream's
    small per-iteration gaps then hold it.
  * Each early dma_start issued on the scalar ring costs ~3.5us of ACT
    sequencer time, so the scalar ring carries only late y-output DMAs;
    inputs ride the sync ring (priority slices first: first 512 xkv cols,
    first 1024 xq cols, weights at the head) and the gpsimd SWDGE queue.
  * The remaining k|v / q projections are woven into the chunk-0 stream at
    at most one matmul-pair per j (PE executes in order; heavier weaves
    starve the exp stream 1:1, lighter ones let the p-state drop).
  * v^T is produced per-1024-column piece: fused k|v projection, DVE
    drain, and four piece-granular hardware DMA transposes — the first AV
    only waits for piece 0, so only PRE=4 exps need banking and the
    end-of-stream AV debt (the kernel tail) stays small.
  * The AV stationary is trimmed to 65 columns (64 v^T + ones).
"""

import numpy as np
import ml_dtypes

import concourse.bacc as bacc
import concourse.mybir as mybir
import concourse.tile as tile
from concourse.bass_utils import run_bass_kernel_spmd

F32 = mybir.dt.float32
BF16 = mybir.dt.bfloat16

B, C, HGT, WID = 2, 256, 64, 64
S = HGT * WID  # 4096 pixels
NH, HD = 4, 64
NCORES = 8
P = 128
IC = 1024  # i-chunk width (2 PSUM banks)
NI = S // IC  # 4
NJ = S // P  # 32 j-blocks
SCALE = HD ** -0.5
KPRI = 512   # priority xkv columns (first k|v projection slice)
QPRI = 1024  # priority xq columns (chunk-0 q projection)
PRE = 6      # chunk-0 exps banked ahead of the first AV (covers the
             # first v-transpose piece)


def _emit(tc):
    nc = tc.nc
    xq = nc.dram_tensor("xq", [2, P, S], BF16, kind="ExternalInput").ap()
    xkv = nc.dram_tensor("xkv", [2, P, S], BF16, kind="ExternalInput").ap()
    wqT = nc.dram_tensor("wqT", [2, P, HD], BF16, kind="ExternalInput").ap()
    wkvT = nc.dram_tensor("wkvT", [2, P, P], BF16, kind="ExternalInput").ap()
    woT = nc.dram_tensor("woT", [HD, C], BF16, kind="ExternalInput").ap()
    bq = nc.dram_tensor("bq", [HD, 1], F32, kind="ExternalInput").ap()
    y = nc.dram_tensor("y", [2, P, S], BF16, kind="ExternalOutput").ap()
    yden = nc.dram_tensor("yden", [1, S], F32, kind="ExternalOutput").ap()

    with (
        tc.tile_pool(name="const", bufs=1) as cpool,
        tc.tile_pool(name="xp", bufs=1) as xpool,
        tc.tile_pool(name="qkv", bufs=1) as qpool,
        tc.tile_pool(name="es", bufs=10) as epool,
        tc.tile_pool(name="epi", bufs=2) as fpool,
        tc.tile_pool(name="ps", bufs=2, space="PSUM") as pp,
    ):
        # ---- critical-path weights + priority slices head the sync ring --
        wkv_sb = cpool.tile([P, 2 * P], BF16)
        wq_sb = cpool.tile([P, 2 * HD], BF16)
        bq_sb = cpool.tile([HD, 1], F32)
        for cch in range(2):
            nc.sync.dma_start(wkv_sb[:, cch * P:(cch + 1) * P], wkvT[cch])
        for cch in range(2):
            nc.sync.dma_start(wq_sb[:, cch * HD:(cch + 1) * HD], wqT[cch])
        nc.sync.dma_start(bq_sb[:], bq)

        xq_sb = [xpool.tile([P, S], BF16, tag=f"xq{i}", name=f"xq_sb{i}")
                 for i in range(2)]
        xkv_sb = [xpool.tile([P, S], BF16, tag=f"xkv{i}", name=f"xkv_sb{i}")
                  for i in range(2)]
        # Inputs ride sync + SWDGE only: early dma_starts on the scalar
        # ring cost ~3.5us of ACT sequencer each, straight off the pacer.
        nc.sync.dma_start(xkv_sb[1][:, 0:KPRI], xkv[1][:, 0:KPRI])
        nc.sync.dma_start(xkv_sb[0][:, 0:KPRI], xkv[0][:, 0:KPRI])
        for s in range(1, 5):
            sl = slice(s * 512, (s + 1) * 512)
            nc.sync.dma_start(xkv_sb[0][:, sl], xkv[0][:, sl])

        # ---- the rest on the gpsimd SWDGE queue (self-issued; starts
        # ~8us in and moves pieces faster than the sync ring, so both
        # priority xq halves ride at its head) ----
        nc.gpsimd.dma_start(xq_sb[1][:, 0:QPRI], xq[1][:, 0:QPRI])
        nc.gpsimd.dma_start(xq_sb[0][:, 0:QPRI], xq[0][:, 0:QPRI])
        wo_sb = cpool.tile([HD, C], BF16)
        nc.gpsimd.dma_start(wo_sb[:], woT)
        for s in range(1, 5):
            sl = slice(s * 512, (s + 1) * 512)
            nc.gpsimd.dma_start(xkv_sb[1][:, sl], xkv[1][:, sl])
        for s in range(5, S // 512):
            sl = slice(s * 512, (s + 1) * 512)
            nc.gpsimd.dma_start(xkv_sb[1][:, sl], xkv[1][:, sl])
            nc.gpsimd.dma_start(xkv_sb[0][:, sl], xkv[0][:, sl])
        for t in range(1, S // 1024):
            sl = slice(t * 1024, (t + 1) * 1024)
            nc.gpsimd.dma_start(xq_sb[1][:, sl], xq[1][:, sl])

        # Zero bias for exp via memset (a float bias would become a DMA'd
        # const tensor queued behind the input DMAs).
        zbias_sb = cpool.tile([P, 1], F32)
        nc.vector.memset(zbias_sb[:], 0.0)

        # PE warmup burst: ~10us of dense matmuls while the input DMAs are
        # in flight; the activity monitor promotes the PE to 2.4GHz after
        # ~5us of sustained streaming, right before the first projection.
        wrm_sb = cpool.tile([P, 512], BF16)
        nc.vector.memset(wrm_sb[:], 0.0)
        for w in range(12):
            wp = pp.tile([P, 512], F32, tag="st", bufs=2, name="wp")
            nc.tensor.matmul(wp[:], wrm_sb[:, 0:P], wrm_sb[:],
                             start=True, stop=True)
        # Warmup exp so the ~2.7us activation-table load happens before the
        # first real exp.
        warm_sb = cpool.tile([P, 1], BF16)
        nc.scalar.activation(warm_sb[:], zbias_sb[:],
                             mybir.ActivationFunctionType.Exp,
                             bias=zbias_sb[:])

        # q/k zero-padded to 128 partitions (the zero rows contribute
        # nothing to the contraction). Only the slices the first exps need
        # are zeroed up front: a full-width [64,4096] DVE memset costs
        # ~3.5us and would queue ahead of the projection drains, gating the
        # first exp; the rest is deferred until the exp stream is running.
        q_sb = qpool.tile([P, S], BF16)
        k_sb = qpool.tile([P, S], BF16)
        nc.vector.memset(k_sb[HD:P, 0:KPRI], 0.0)
        nc.vector.memset(q_sb[HD:P, 0:QPRI], 0.0)
        # v (dense, pre-transpose) lives on partitions 64:128 (the fused
        # k|v projection's PSUM rows), ready for the hardware transpose.
        v_sb = qpool.tile([P, S], BF16)
        # v^T blocks [j-part, (block, 128)]: cols 0:64 = v^T (the hardware
        # transpose requires the 128-stride block layout), col 64 = ones.
        # The AV stationary slices only cols 0:65.
        va_sb = qpool.tile([P, NJ * P], BF16)
        va_v = va_sb.rearrange("p (j c) -> p j c", c=P)
        nc.vector.memset(va_v[:, :, HD:HD + 1], 1.0)

        # ---- projections ----
        def kv_proj(s):
            # fused: stationary (wk^T | wv^T) -> PSUM rows 0:64 = k,
            # rows 64:128 = v, one matmul pass per 512-column slice
            sl = slice(s * 512, (s + 1) * 512)
            kvp = pp.tile([P, 512], F32, tag="av", bufs=2, name="kvp")
            nc.tensor.matmul(kvp[:], wkv_sb[:, 0:P], xkv_sb[0][:, sl],
                             start=True, stop=False)
            nc.tensor.matmul(kvp[:], wkv_sb[:, P:2 * P], xkv_sb[1][:, sl],
                             start=False, stop=True)
            nc.vector.tensor_copy(k_sb[0:HD, sl], kvp[0:HD, :])
            nc.vector.tensor_copy(v_sb[HD:P, sl], kvp[HD:P, :])

        def q_proj(t):
            sl = slice(t * 512, (t + 1) * 512)
            qp = pp.tile([HD, 512], F32, tag="av", bufs=2, name="qp")
            nc.tensor.matmul(qp[:], wq_sb[:, 0:HD], xq_sb[0][:, sl],
                             start=True, stop=False)
            nc.tensor.matmul(qp[:], wq_sb[:, HD:2 * HD], xq_sb[1][:, sl],
                             start=False, stop=True)
            nc.vector.tensor_scalar_add(q_sb[0:HD, sl], qp[:], bq_sb[:])

        kv_proj(0)
        q_proj(0)
        q_proj(1)
        # deferred zero-pad remainders: DVE runs these ~16-22us while the
        # exp stream is already going; k columns 512:640 (j-block 4) are
        # needed first, at ~21us
        nc.vector.memset(k_sb[HD:P, KPRI:S], 0.0)
        nc.vector.memset(q_sb[HD:P, QPRI:S], 0.0)

        def transpose_piece(g):
            # v^T for j-blocks 8g..8g+7, available as soon as v slices
            # 2g/2g+1 are drained — the first AV only needs piece 0.
            nc.sync.dma_start_transpose(
                out=va_v[:, 8 * g:8 * (g + 1), 0:HD],
                in_=v_sb[HD:P, 1024 * g:1024 * (g + 1)])

        def xq0_rest():
            for t in range(1, S // 1024):
                sl = slice(t * 1024, (t + 1) * 1024)
                nc.sync.dma_start(xq_sb[0][:, sl], xq[0][:, sl])

        # ---- attention ----
        def qk_exp(c, j):
            st = pp.tile([P, IC], F32, tag="st", bufs=2, name="st")
            for h in range(IC // 512):
                isl = slice(c * IC + h * 512, c * IC + (h + 1) * 512)
                nc.tensor.matmul(st[:, h * 512:(h + 1) * 512],
                                 k_sb[:, j * P:(j + 1) * P],
                                 q_sb[:, isl],
                                 start=True, stop=True)
            et = epool.tile([P, IC], BF16, name="et")
            nc.scalar.activation(et[:], st[:],
                                 mybir.ActivationFunctionType.Exp,
                                 bias=zbias_sb[:])
            return et

        pend = [None] * NI

        def epilogue_part2(i, final=False, ohs=(0, 1)):
            outt = pend[i]
            for oh in ohs:
                for h in range(IC // 512):
                    yp = pp.tile([P, 512], F32, tag="av", bufs=2, name="yp")
                    nc.tensor.matmul(yp[:], wo_sb[:, oh * P:(oh + 1) * P],
                                     outt[:, h * 512:(h + 1) * 512],
                                     start=True, stop=True)
                    ys = fpool.tile([P, 512], BF16, name="ys")
                    if final and (oh + h) % 2 == 1:
                        nc.scalar.activation(
                            ys[:], yp[:], mybir.ActivationFunctionType.Copy)
                    else:
                        nc.vector.tensor_copy(ys[:], yp[:])
                    eng = nc.sync if oh == 0 else nc.scalar
                    eng.dma_start(
                        y[oh][:, i * IC + h * 512:i * IC + (h + 1) * 512],
                        ys[:])

        # Chunk-0 weave: remaining projections + transpose pieces ride the
        # exp-paced stream at at most one matmul-pair per j.
        weave0 = {
            0: [lambda: kv_proj(1)],
            1: [lambda: transpose_piece(0)],
            2: [lambda: kv_proj(2)],
            4: [lambda: kv_proj(3)],
            5: [lambda: transpose_piece(1)],
            6: [lambda: kv_proj(4)],
            8: [lambda: kv_proj(5)],
            9: [lambda: transpose_piece(2)],
            10: [lambda: kv_proj(6)],
            12: [lambda: kv_proj(7)],
            13: [lambda: transpose_piece(3)],
            14: [lambda: xq0_rest()],
            15: [lambda: q_proj(2)],
            17: [lambda: q_proj(3)],
            19: [lambda: q_proj(4)],
            21: [lambda: q_proj(5)],
            23: [lambda: q_proj(6)],
            25: [lambda: q_proj(7)],
        }

        bank = []
        for j in range(PRE):
            bank.append(qk_exp(0, j))
            for fn in weave0.get(j, []):
                fn()

        for i in range(NI):
            av = pp.tile([HD + 1, IC], F32, tag="av", bufs=2, name="av")
            for j in range(NJ):
                if i > 0 and j == 8:
                    epilogue_part2(i - 1, ohs=(0,))
                if i > 0 and j == 10:
                    epilogue_part2(i - 1, ohs=(1,))
                if i == 0 and j < PRE:
                    et = bank[j]
                else:
                    et = qk_exp(i, j)
                    if i == 0:
                        for fn in weave0.get(j, []):
                            fn()
                for h in (1, 0):
                    nc.tensor.matmul(av[:, h * 512:(h + 1) * 512],
                                     va_v[:, j, 0:HD + 1],
                                     et[:, h * 512:(h + 1) * 512],
                                     start=(j == 0), stop=(j == NJ - 1))

            outt = fpool.tile([HD, IC], BF16, name="outt")
            if i == NI - 1:
                nc.vector.tensor_copy(outt[:, 0:512], av[0:HD, 0:512])
                nc.vector.tensor_copy(outt[:, 512:IC], av[0:HD, 512:IC])
            else:
                nc.vector.tensor_copy(outt[:], av[0:HD, :])
            den = fpool.tile([1, IC], F32, name="den")
            nc.vector.tensor_copy(den[:], av[HD:HD + 1, :])
            nc.gpsimd.dma_start(yden[:, i * IC:(i + 1) * IC], den[:])
            pend[i] = outt

        epilogue_part2(NI - 1, final=True)


def build():
    nc = bacc.Bacc("TRN2", target_bir_lowering=False, debug=False,
                   enable_asserts=False)
    with tile.TileContext(nc) as tc:
        _emit(tc)
    nc.compile()
    return nc


_NC_CACHE = []


def _get_nc():
    if not _NC_CACHE:
        _NC_CACHE.append(build())
    return _NC_CACHE[0]


def make_in_maps(x_q, x_kv, wq, bq, wk, bk, wv, bv, wo, bo):
    bf = ml_dtypes.bfloat16
    in_maps = []
    bo_effs = []
    for c in range(NCORES):
        b, n = divmod(c, NH)
        hs = slice(n * HD, (n + 1) * HD)
        wq_h = wq[hs].astype(np.float64) * SCALE
        bo_eff = wo[:, hs].astype(np.float64) @ bv[hs].astype(np.float64)
        if n == 0:
            bo_eff = bo_eff + bo.astype(np.float64)
        bo_effs.append(bo_eff.astype(np.float32))
        in_maps.append({
            "xq": np.ascontiguousarray(
                x_q[b].reshape(C, S).reshape(2, P, S)).astype(bf),
            "xkv": np.ascontiguousarray(
                x_kv[b].reshape(C, S).reshape(2, P, S)).astype(bf),
            "wqT": np.ascontiguousarray(wq_h.T.reshape(2, P, HD)).astype(bf),
            "wkvT": np.ascontiguousarray(
                np.concatenate([wk[hs].T, wv[hs].T], axis=1)
                .reshape(2, P, P)).astype(bf),
            "woT": np.ascontiguousarray(wo[:, hs].T).astype(bf),
            "bq": (bq[hs].astype(np.float64) * SCALE
                   ).astype(np.float32).reshape(HD, 1),
        })
    return in_maps, bo_effs


def assemble_output(results, bo_effs):
    # y_core is the unnormalized head partial; divide by the softmax
    # denominator and add the (host-folded) bias here.
    y = np.zeros((B, C, S), np.float32)
    for c in range(NCORES):
        b = c // NH
        den = results[c]["yden"].reshape(1, S)
        y[b] += results[c]["y"].astype(np.float32).reshape(C, S) / den \
            + bo_effs[c].reshape(C, 1)
    return y.reshape(B, C, HGT, WID)


def kernel(**inputs):
    nc = _get_nc()
    in_maps, bo_effs = make_in_maps(**inputs)
    res = run_bass_kernel_spmd(nc, in_maps, list(range(NCORES)))
    return assemble_output(res.results, bo_effs)


if __name__ == "__main__":
    nc = build()
    print("built + compiled ok")


# revision 34
# speedup vs baseline: 1.1490x; 1.0051x over previous
"""Cross-attention kernel for Trainium2, sharded over 8 NeuronCores.

Problem (per reference):
  q = wq @ x_q + bq ; k = wk @ x_kv + bk ; v = wv @ x_kv + bv   (1x1 convs)
  per head: attn = softmax(q^T k / sqrt(hd)) ; out = attn @ v^T
  y = wo @ out + bo

Sharding: core c -> (batch b = c // 4, head n = c % 4). Each core runs one
head's full attention and produces the partial output projection
y_part = wo[:, head] @ out_head; the host sums the 4 head partials per batch.

Mathematically exact simplifications (as in the 192us version):
  * bk drops (per-query logit shift cancels in softmax); bv folds into the
    host-side bias (softmax rows sum to 1); scale folds into wq/bq;
    no max-subtraction (logits ~N(0,1)); softmax denominator comes from a
    ones-column in the AV stationary; normalization deferred to the host
    (ships y_un + per-pixel denominators, host divides).

Speed structure. The scalar engine's exp stream is the pacer: 128 exp
instructions of [128,1024] at ~1.11us each (1 elem/cycle/partition at
1.2GHz + ~230ns instruction overhead) ~= 142us; everything else must hide
under it. Measured facts this schedule is built on (NTFF traces):
  * A 512-col matmul costs ~215ns streaming at the promoted 2.4GHz PE
    clock + ~100-135ns weight load; per attention iteration the PE runs
    QK(2) + AV(2) matmuls ~= 1.05-1.25us, just under the exp. fp8
    DoubleRow gives NO streaming speedup on this silicon (tried: 512-out
    DoubleRow measures ~375ns like bf16), so everything stays bf16.
  * The PE p-state: ~5us of continuous warmup matmuls promote 1.2->2.4GHz
    (~13us in, right before the first projection); the exp-paced stream's
    small per-iteration gaps then hold it.
  * Each early dma_start issued on the scalar ring costs ~3.5us of ACT
    sequencer time, so the scalar ring carries only late y-output DMAs;
    inputs ride the sync ring (priority slices first: first 512 xkv cols,
    first 1024 xq cols, weights at the head) and the gpsimd SWDGE queue.
  * The remaining k|v / q projections are woven into the chunk-0 stream at
    at most one matmul-pair per j (PE executes in order; heavier weaves
    starve the exp stream 1:1, lighter ones let the p-state drop).
  * v^T is produced per-1024-column piece: fused k|v projection, DVE
    drain, and four piece-granular hardware DMA transposes — the first AV
    only waits for piece 0, so only PRE=4 exps need banking and the
    end-of-stream AV debt (the kernel tail) stays small.
  * The AV stationary is trimmed to 65 columns (64 v^T + ones).
"""

import numpy as np
import ml_dtypes

import concourse.bacc as bacc
import concourse.mybir as mybir
import concourse.tile as tile
from concourse.bass_utils import run_bass_kernel_spmd

F32 = mybir.dt.float32
BF16 = mybir.dt.bfloat16

B, C, HGT, WID = 2, 256, 64, 64
S = HGT * WID  # 4096 pixels
NH, HD = 4, 64
NCORES = 8
P = 128
IC = 1024  # i-chunk width (2 PSUM banks)
NI = S // IC  # 4
NJ = S // P  # 32 j-blocks
SCALE = HD ** -0.5
KPRI = 512   # priority xkv columns (first k|v projection slice)
QPRI = 1024  # priority xq columns (chunk-0 q projection)
PRE = 5      # chunk-0 exps banked ahead of the first AV (covers the
             # first v-transpose piece)


def _emit(tc):
    nc = tc.nc
    xq = nc.dram_tensor("xq", [2, P, S], BF16, kind="ExternalInput").ap()
    xkv = nc.dram_tensor("xkv", [2, P, S], BF16, kind="ExternalInput").ap()
    # wblob packs (wk|wv) [128 cols] + wq [64 cols] per channel half: the
    # critical-path weights arrive in ONE transfer (each DMA piece carries
    # ~1.3us of fixed cost on the ring — small pieces starve the prologue)
    wblob = nc.dram_tensor("wblob", [2, P, P + HD], BF16,
                           kind="ExternalInput").ap()
    woT = nc.dram_tensor("woT", [HD, C], BF16, kind="ExternalInput").ap()
    bq = nc.dram_tensor("bq", [HD, 1], F32, kind="ExternalInput").ap()
    y = nc.dram_tensor("y", [2, P, S], BF16, kind="ExternalOutput").ap()
    yden = nc.dram_tensor("yden", [1, S], F32, kind="ExternalOutput").ap()

    with (
        tc.tile_pool(name="const", bufs=1) as cpool,
        tc.tile_pool(name="xp", bufs=1) as xpool,
        tc.tile_pool(name="qkv", bufs=1) as qpool,
        tc.tile_pool(name="es", bufs=10) as epool,
        tc.tile_pool(name="epi", bufs=2) as fpool,
        tc.tile_pool(name="ps", bufs=2, space="PSUM") as pp,
    ):
        # ---- critical-path weights + priority slices head the sync ring,
        # consolidated into FEW LARGE transfers (per-piece fixed cost
        # ~1.3us; 12 small pieces pushed the priority data past the
        # warmup's end and let the PE p-state demote) ----
        wb_sb = cpool.tile([P, 2 * (P + HD)], BF16)
        wb_v = wb_sb.rearrange("p (c w) -> p c w", c=2)
        wkv_sb = [wb_v[:, cch, 0:P] for cch in range(2)]
        wq_sb = [wb_v[:, cch, P:P + HD] for cch in range(2)]
        bq_sb = cpool.tile([HD, 1], F32)
        nc.sync.dma_start(wb_v[:, :, :], wblob.rearrange("c p w -> p c w"))
        nc.sync.dma_start(bq_sb[:], bq)

        # activations as single tiles [p, (half, col)] so one DMA can carry
        # both channel halves of a column range; DRAM-side APs are
        # rearranged to (p, half, col) so source and dest walk in the same
        # dimension order
        xq_sb = xpool.tile([P, 2 * S], BF16)
        xq_v = xq_sb.rearrange("p (c w) -> p c w", c=2)
        xkv_sb = xpool.tile([P, 2 * S], BF16)
        xkv_v = xkv_sb.rearrange("p (c w) -> p c w", c=2)
        xq_r = xq.rearrange("c p w -> p c w")
        xkv_r = xkv.rearrange("c p w -> p c w")
        # Inputs ride sync + SWDGE only: early dma_starts on the scalar
        # ring cost ~3.5us of ACT sequencer each, straight off the pacer.
        # The first-exp chain splits across the two queues: sync carries
        # xq half 0, SWDGE carries xq half 1 + the xkv priority columns.
        nc.sync.dma_start(xq_v[:, 0, 0:QPRI], xq[0][:, 0:QPRI])
        nc.sync.dma_start(xkv_v[:, 0, 512:1536], xkv[0][:, 512:1536])
        nc.sync.dma_start(xkv_v[:, 0, 1536:2560], xkv[0][:, 1536:2560])

        # ---- the rest on the gpsimd SWDGE queue (self-issued) ----
        nc.gpsimd.dma_start(xq_v[:, 1, 0:QPRI], xq[1][:, 0:QPRI])
        nc.gpsimd.dma_start(xkv_v[:, :, 0:KPRI], xkv_r[:, :, 0:KPRI])
        nc.gpsimd.dma_start(xkv_v[:, 1, 512:1536], xkv[1][:, 512:1536])
        wo_sb = cpool.tile([HD, C], BF16)
        nc.gpsimd.dma_start(wo_sb[:], woT)
        nc.gpsimd.dma_start(xkv_v[:, 1, 1536:2560], xkv[1][:, 1536:2560])
        nc.gpsimd.dma_start(xkv_v[:, 1, 2560:S], xkv[1][:, 2560:S])
        nc.gpsimd.dma_start(xkv_v[:, 0, 2560:S], xkv[0][:, 2560:S])
        for t in range(1, S // 1024):
            sl = slice(t * 1024, (t + 1) * 1024)
            nc.gpsimd.dma_start(xq_v[:, :, sl], xq_r[:, :, sl])

        # Zero bias for exp via memset (a float bias would become a DMA'd
        # const tensor queued behind the input DMAs).
        zbias_sb = cpool.tile([P, 1], F32)
        nc.vector.memset(zbias_sb[:], 0.0)

        # PE warmup burst: ~10us of dense matmuls while the input DMAs are
        # in flight; the activity monitor promotes the PE to 2.4GHz after
        # ~5us of sustained streaming, right before the first projection.
        wrm_sb = cpool.tile([P, 512], BF16)
        nc.vector.memset(wrm_sb[:], 0.0)
        for w in range(12):
            wp = pp.tile([P, 512], F32, tag="st", bufs=2, name="wp")
            nc.tensor.matmul(wp[:], wrm_sb[:, 0:P], wrm_sb[:],
                             start=True, stop=True)
        # Warmup exp so the ~2.7us activation-table load happens before the
        # first real exp.
        warm_sb = cpool.tile([P, 1], BF16)
        nc.scalar.activation(warm_sb[:], zbias_sb[:],
                             mybir.ActivationFunctionType.Exp,
                             bias=zbias_sb[:])

        # q/k zero-padded to 128 partitions (the zero rows contribute
        # nothing to the contraction). Only the slices the first exps need
        # are zeroed up front: a full-width [64,4096] DVE memset costs
        # ~3.5us and would queue ahead of the projection drains, gating the
        # first exp; the rest is deferred until the exp stream is running.
        q_sb = qpool.tile([P, S], BF16)
        k_sb = qpool.tile([P, S], BF16)
        nc.vector.memset(k_sb[HD:P, 0:KPRI], 0.0)
        nc.vector.memset(q_sb[HD:P, 0:QPRI], 0.0)
        # v (dense, pre-transpose) lives on partitions 64:128 (the fused
        # k|v projection's PSUM rows), ready for the hardware transpose.
        v_sb = qpool.tile([P, S], BF16)
        # v^T blocks [j-part, (block, 128)]: cols 0:64 = v^T (the hardware
        # transpose requires the 128-stride block layout), col 64 = ones.
        # The AV stationary slices only cols 0:65.
        va_sb = qpool.tile([P, NJ * P], BF16)
        va_v = va_sb.rearrange("p (j c) -> p j c", c=P)
        nc.vector.memset(va_v[:, :, HD:HD + 1], 1.0)

        # ---- projections ----
        def kv_proj(s):
            # fused: stationary (wk^T | wv^T) -> PSUM rows 0:64 = k,
            # rows 64:128 = v, one matmul pass per 512-column slice
            sl = slice(s * 512, (s + 1) * 512)
            kvp = pp.tile([P, 512], F32, tag="av", bufs=2, name="kvp")
            nc.tensor.matmul(kvp[:], wkv_sb[0], xkv_v[:, 0, sl],
                             start=True, stop=False)
            nc.tensor.matmul(kvp[:], wkv_sb[1], xkv_v[:, 1, sl],
                             start=False, stop=True)
            nc.vector.tensor_copy(k_sb[0:HD, sl], kvp[0:HD, :])
            nc.vector.tensor_copy(v_sb[HD:P, sl], kvp[HD:P, :])

        def q_proj(t):
            sl = slice(t * 512, (t + 1) * 512)
            qp = pp.tile([HD, 512], F32, tag="av", bufs=2, name="qp")
            nc.tensor.matmul(qp[:], wq_sb[0], xq_v[:, 0, sl],
                             start=True, stop=False)
            nc.tensor.matmul(qp[:], wq_sb[1], xq_v[:, 1, sl],
                             start=False, stop=True)
            nc.vector.tensor_scalar_add(q_sb[0:HD, sl], qp[:], bq_sb[:])

        kv_proj(0)
        q_proj(0)
        q_proj(1)
        # deferred zero-pad remainders: DVE runs these ~16-22us while the
        # exp stream is already going; k columns 512:640 (j-block 4) are
        # needed first, at ~21us
        nc.vector.memset(k_sb[HD:P, KPRI:S], 0.0)
        nc.vector.memset(q_sb[HD:P, QPRI:S], 0.0)

        def transpose_piece(g):
            # v^T for j-blocks 8g..8g+7, available as soon as v slices
            # 2g/2g+1 are drained — the first AV only needs piece 0.
            nc.sync.dma_start_transpose(
                out=va_v[:, 8 * g:8 * (g + 1), 0:HD],
                in_=v_sb[HD:P, 1024 * g:1024 * (g + 1)])

        # ---- attention ----
        def qk_exp(c, j):
            st = pp.tile([P, IC], F32, tag="st", bufs=2, name="st")
            for h in range(IC // 512):
                isl = slice(c * IC + h * 512, c * IC + (h + 1) * 512)
                nc.tensor.matmul(st[:, h * 512:(h + 1) * 512],
                                 k_sb[:, j * P:(j + 1) * P],
                                 q_sb[:, isl],
                                 start=True, stop=True)
            et = epool.tile([P, IC], BF16, name="et")
            nc.scalar.activation(et[:], st[:],
                                 mybir.ActivationFunctionType.Exp,
                                 bias=zbias_sb[:])
            return et

        pend = [None] * NI

        def epilogue_part2(i, final=False, ohs=(0, 1)):
            outt = pend[i]
            for oh in ohs:
                for h in range(IC // 512):
                    yp = pp.tile([P, 512], F32, tag="av", bufs=2, name="yp")
                    nc.tensor.matmul(yp[:], wo_sb[:, oh * P:(oh + 1) * P],
                                     outt[:, h * 512:(h + 1) * 512],
                                     start=True, stop=True)
                    ys = fpool.tile([P, 512], BF16, name="ys")
                    if final and (oh + h) % 2 == 1:
                        nc.scalar.activation(
                            ys[:], yp[:], mybir.ActivationFunctionType.Copy)
                    else:
                        nc.vector.tensor_copy(ys[:], yp[:])
                    eng = nc.sync if oh == 0 else nc.scalar
                    eng.dma_start(
                        y[oh][:, i * IC + h * 512:i * IC + (h + 1) * 512],
                        ys[:])

        # Chunk-0 weave: remaining projections + transpose pieces ride the
        # exp-paced stream at at most one matmul-pair per j.
        weave0 = {
            0: [lambda: kv_proj(1)],
            1: [lambda: transpose_piece(0)],
            2: [lambda: kv_proj(2)],
            4: [lambda: kv_proj(3)],
            5: [lambda: transpose_piece(1)],
            6: [lambda: kv_proj(4)],
            8: [lambda: kv_proj(5)],
            9: [lambda: transpose_piece(2)],
            10: [lambda: kv_proj(6)],
            12: [lambda: kv_proj(7)],
            13: [lambda: transpose_piece(3)],
            15: [lambda: q_proj(2)],
            17: [lambda: q_proj(3)],
            19: [lambda: q_proj(4)],
            21: [lambda: q_proj(5)],
            23: [lambda: q_proj(6)],
            25: [lambda: q_proj(7)],
        }

        bank = []
        for j in range(PRE):
            bank.append(qk_exp(0, j))
            for fn in weave0.get(j, []):
                fn()

        for i in range(NI):
            av = pp.tile([HD + 1, IC], F32, tag="av", bufs=2, name="av")
            for j in range(NJ):
                if i > 0 and j == 8:
                    epilogue_part2(i - 1, ohs=(0,))
                if i > 0 and j == 10:
                    epilogue_part2(i - 1, ohs=(1,))
                if i == 0 and j < PRE:
                    et = bank[j]
                else:
                    et = qk_exp(i, j)
                    if i == 0:
                        for fn in weave0.get(j, []):
                            fn()
                for h in (1, 0):
                    nc.tensor.matmul(av[:, h * 512:(h + 1) * 512],
                                     va_v[:, j, 0:HD + 1],
                                     et[:, h * 512:(h + 1) * 512],
                                     start=(j == 0), stop=(j == NJ - 1))

            outt = fpool.tile([HD, IC], BF16, name="outt")
            if i == NI - 1:
                nc.vector.tensor_copy(outt[:, 0:512], av[0:HD, 0:512])
                nc.vector.tensor_copy(outt[:, 512:IC], av[0:HD, 512:IC])
            else:
                nc.vector.tensor_copy(outt[:], av[0:HD, :])
            den = fpool.tile([1, IC], F32, name="den")
            nc.vector.tensor_copy(den[:], av[HD:HD + 1, :])
            nc.gpsimd.dma_start(yden[:, i * IC:(i + 1) * IC], den[:])
            pend[i] = outt

        epilogue_part2(NI - 1, final=True)


def build():
    nc = bacc.Bacc("TRN2", target_bir_lowering=False, debug=False,
                   enable_asserts=False)
    with tile.TileContext(nc) as tc:
        _emit(tc)
    nc.compile()
    return nc


_NC_CACHE = []


def _get_nc():
    if not _NC_CACHE:
        _NC_CACHE.append(build())
    return _NC_CACHE[0]


def make_in_maps(x_q, x_kv, wq, bq, wk, bk, wv, bv, wo, bo):
    bf = ml_dtypes.bfloat16
    in_maps = []
    bo_effs = []
    for c in range(NCORES):
        b, n = divmod(c, NH)
        hs = slice(n * HD, (n + 1) * HD)
        wq_h = wq[hs].astype(np.float64) * SCALE
        bo_eff = wo[:, hs].astype(np.float64) @ bv[hs].astype(np.float64)
        if n == 0:
            bo_eff = bo_eff + bo.astype(np.float64)
        bo_effs.append(bo_eff.astype(np.float32))
        in_maps.append({
            "xq": np.ascontiguousarray(
                x_q[b].reshape(C, S).reshape(2, P, S)).astype(bf),
            "xkv": np.ascontiguousarray(
                x_kv[b].reshape(C, S).reshape(2, P, S)).astype(bf),
            "wblob": np.ascontiguousarray(
                np.concatenate(
                    [np.concatenate([wk[hs].T, wv[hs].T], axis=1)
                     .reshape(2, P, P),
                     wq_h.T.reshape(2, P, HD)], axis=2)).astype(bf),
            "woT": np.ascontiguousarray(wo[:, hs].T).astype(bf),
            "bq": (bq[hs].astype(np.float64) * SCALE
                   ).astype(np.float32).reshape(HD, 1),
        })
    return in_maps, bo_effs


def assemble_output(results, bo_effs):
    # y_core is the unnormalized head partial; divide by the softmax
    # denominator and add the (host-folded) bias here.
    y = np.zeros((B, C, S), np.float32)
    for c in range(NCORES):
        b = c // NH
        den = results[c]["yden"].reshape(1, S)
        y[b] += results[c]["y"].astype(np.float32).reshape(C, S) / den \
            + bo_effs[c].reshape(C, 1)
    return y.reshape(B, C, HGT, WID)


def kernel(**inputs):
    nc = _get_nc()
    in_maps, bo_effs = make_in_maps(**inputs)
    res = run_bass_kernel_spmd(nc, in_maps, list(range(NCORES)))
    return assemble_output(res.results, bo_effs)


if __name__ == "__main__":
    nc = build()
    print("built + compiled ok")


# revision 38
# speedup vs baseline: 1.1516x; 1.0023x over previous
"""Cross-attention kernel for Trainium2, sharded over 8 NeuronCores.

Problem (per reference):
  q = wq @ x_q + bq ; k = wk @ x_kv + bk ; v = wv @ x_kv + bv   (1x1 convs)
  per head: attn = softmax(q^T k / sqrt(hd)) ; out = attn @ v^T
  y = wo @ out + bo

Sharding: core c -> (batch b = c // 4, head n = c % 4). Each core runs one
head's full attention and produces the partial output projection
y_part = wo[:, head] @ out_head; the host sums the 4 head partials per batch.

Mathematically exact simplifications (as in the 192us version):
  * bk drops (per-query logit shift cancels in softmax); bv folds into the
    host-side bias (softmax rows sum to 1); scale folds into wq/bq;
    no max-subtraction (logits ~N(0,1)); softmax denominator comes from a
    ones-column in the AV stationary; normalization deferred to the host
    (ships y_un + per-pixel denominators, host divides).

Speed structure. The scalar engine's exp stream is the pacer: 128 exp
instructions of [128,1024] at ~1.11us each (1 elem/cycle/partition at
1.2GHz + ~230ns instruction overhead) ~= 142us; everything else must hide
under it. Measured facts this schedule is built on (NTFF traces):
  * A 512-col matmul costs ~215ns streaming at the promoted 2.4GHz PE
    clock + ~100-135ns weight load; per attention iteration the PE runs
    QK(2) + AV(2) matmuls ~= 1.05-1.25us, just under the exp. fp8
    DoubleRow gives NO streaming speedup on this silicon (tried: 512-out
    DoubleRow measures ~375ns like bf16), so everything stays bf16.
  * The PE p-state: ~5us of continuous warmup matmuls promote 1.2->2.4GHz
    (~13us in, right before the first projection); the exp-paced stream's
    small per-iteration gaps then hold it.
  * Each early dma_start issued on the scalar ring costs ~3.5us of ACT
    sequencer time, so the scalar ring carries only late y-output DMAs;
    inputs ride the sync ring (priority slices first: first 512 xkv cols,
    first 1024 xq cols, weights at the head) and the gpsimd SWDGE queue.
  * The remaining k|v / q projections are woven into the chunk-0 stream at
    at most one matmul-pair per j (PE executes in order; heavier weaves
    starve the exp stream 1:1, lighter ones let the p-state drop).
  * v^T is produced per-1024-column piece: fused k|v projection, DVE
    drain, and four piece-granular hardware DMA transposes — the first AV
    only waits for piece 0, so only PRE=4 exps need banking and the
    end-of-stream AV debt (the kernel tail) stays small.
  * The AV stationary is trimmed to 65 columns (64 v^T + ones).
"""

import numpy as np
import ml_dtypes

import concourse.bacc as bacc
import concourse.mybir as mybir
import concourse.tile as tile
from concourse.bass_utils import run_bass_kernel_spmd

F32 = mybir.dt.float32
BF16 = mybir.dt.bfloat16

B, C, HGT, WID = 2, 256, 64, 64
S = HGT * WID  # 4096 pixels
NH, HD = 4, 64
NCORES = 8
P = 128
IC = 1024  # i-chunk width (2 PSUM banks)
NI = S // IC  # 4
NJ = S // P  # 32 j-blocks
SCALE = HD ** -0.5
KPRI = 512   # priority xkv columns (first k|v projection slice)
QPRI = 1024  # priority xq columns (chunk-0 q projection)
PRE = 6      # chunk-0 exps banked ahead of the first AV (covers the
             # first v-transpose piece)


def _emit(tc):
    nc = tc.nc
    xq = nc.dram_tensor("xq", [2, P, S], BF16, kind="ExternalInput").ap()
    xkv = nc.dram_tensor("xkv", [2, P, S], BF16, kind="ExternalInput").ap()
    # wblob packs (wk|wv) [128 cols] + wq [64 cols] per channel half: the
    # critical-path weights arrive in ONE transfer (each DMA piece carries
    # ~1.3us of fixed cost on the ring — small pieces starve the prologue)
    wblob = nc.dram_tensor("wblob", [2, P, P + HD], BF16,
                           kind="ExternalInput").ap()
    woT = nc.dram_tensor("woT", [HD, C], BF16, kind="ExternalInput").ap()
    bq = nc.dram_tensor("bq", [HD, 1], F32, kind="ExternalInput").ap()
    y = nc.dram_tensor("y", [2, P, S], BF16, kind="ExternalOutput").ap()
    yden = nc.dram_tensor("yden", [1, S], F32, kind="ExternalOutput").ap()

    with (
        tc.tile_pool(name="const", bufs=1) as cpool,
        tc.tile_pool(name="xp", bufs=1) as xpool,
        tc.tile_pool(name="qkv", bufs=1) as qpool,
        tc.tile_pool(name="es", bufs=10) as epool,
        tc.tile_pool(name="epi", bufs=2) as fpool,
        tc.tile_pool(name="ps", bufs=2, space="PSUM") as pp,
    ):
        # ---- critical-path weights + priority slices head the sync ring,
        # consolidated into FEW LARGE transfers (per-piece fixed cost
        # ~1.3us; 12 small pieces pushed the priority data past the
        # warmup's end and let the PE p-state demote) ----
        wb_sb = cpool.tile([P, 2 * (P + HD)], BF16)
        wb_v = wb_sb.rearrange("p (c w) -> p c w", c=2)
        wkv_sb = [wb_v[:, cch, 0:P] for cch in range(2)]
        wq_sb = [wb_v[:, cch, P:P + HD] for cch in range(2)]
        bq_sb = cpool.tile([HD, 1], F32)
        # wblob rides the scalar ring: its one dma_start costs ~3.5us of
        # ACT sequencer at boot, when ACT is idle anyway, and keeps the
        # sync ring's head free for the xq priority half.
        nc.scalar.dma_start(wb_v[:, :, :], wblob.rearrange("c p w -> p c w"))

        # activations as single tiles [p, (half, col)] so one DMA can carry
        # both channel halves of a column range; DRAM-side APs are
        # rearranged to (p, half, col) so source and dest walk in the same
        # dimension order
        xq_sb = xpool.tile([P, 2 * S], BF16)
        xq_v = xq_sb.rearrange("p (c w) -> p c w", c=2)
        xkv_sb = xpool.tile([P, 2 * S], BF16)
        xkv_v = xkv_sb.rearrange("p (c w) -> p c w", c=2)
        xq_r = xq.rearrange("c p w -> p c w")
        xkv_r = xkv.rearrange("c p w -> p c w")
        # Inputs ride sync + SWDGE only: early dma_starts on the scalar
        # ring cost ~3.5us of ACT sequencer each, straight off the pacer.
        # The first-exp chain splits across the two queues: sync carries
        # xq half 0, SWDGE carries xq half 1 + the xkv priority columns.
        nc.sync.dma_start(xq_v[:, 0, 0:QPRI], xq[0][:, 0:QPRI])
        nc.sync.dma_start(bq_sb[:], bq)
        nc.sync.dma_start(xkv_v[:, 0, 512:1536], xkv[0][:, 512:1536])
        nc.sync.dma_start(xkv_v[:, 0, 1536:2560], xkv[0][:, 1536:2560])

        # ---- the rest on the gpsimd SWDGE queue (self-issued) ----
        nc.gpsimd.dma_start(xq_v[:, 1, 0:QPRI], xq[1][:, 0:QPRI])
        nc.gpsimd.dma_start(xkv_v[:, :, 0:KPRI], xkv_r[:, :, 0:KPRI])
        nc.gpsimd.dma_start(xkv_v[:, 1, 512:1536], xkv[1][:, 512:1536])
        wo_sb = cpool.tile([HD, C], BF16)
        nc.gpsimd.dma_start(wo_sb[:], woT)
        nc.gpsimd.dma_start(xkv_v[:, 1, 1536:2560], xkv[1][:, 1536:2560])
        nc.gpsimd.dma_start(xkv_v[:, 1, 2560:S], xkv[1][:, 2560:S])
        nc.gpsimd.dma_start(xkv_v[:, 0, 2560:S], xkv[0][:, 2560:S])
        for t in range(1, S // 1024):
            sl = slice(t * 1024, (t + 1) * 1024)
            nc.gpsimd.dma_start(xq_v[:, :, sl], xq_r[:, :, sl])

        # Zero bias for exp via memset (a float bias would become a DMA'd
        # const tensor queued behind the input DMAs).
        zbias_sb = cpool.tile([P, 1], F32)
        nc.vector.memset(zbias_sb[:], 0.0)

        # PE warmup burst: ~10us of dense matmuls while the input DMAs are
        # in flight; the activity monitor promotes the PE to 2.4GHz after
        # ~5us of sustained streaming, right before the first projection.
        wrm_sb = cpool.tile([P, 512], BF16)
        nc.vector.memset(wrm_sb[:], 0.0)
        for w in range(12):
            wp = pp.tile([P, 512], F32, tag="st", bufs=2, name="wp")
            nc.tensor.matmul(wp[:], wrm_sb[:, 0:P], wrm_sb[:],
                             start=True, stop=True)
        # Warmup exp so the ~2.7us activation-table load happens before the
        # first real exp.
        warm_sb = cpool.tile([P, 1], BF16)
        nc.scalar.activation(warm_sb[:], zbias_sb[:],
                             mybir.ActivationFunctionType.Exp,
                             bias=zbias_sb[:])

        # q/k zero-padded to 128 partitions (the zero rows contribute
        # nothing to the contraction). Only the slices the first exps need
        # are zeroed up front: a full-width [64,4096] DVE memset costs
        # ~3.5us and would queue ahead of the projection drains, gating the
        # first exp; the rest is deferred until the exp stream is running.
        q_sb = qpool.tile([P, S], BF16)
        k_sb = qpool.tile([P, S], BF16)
        nc.vector.memset(k_sb[HD:P, 0:KPRI], 0.0)
        nc.vector.memset(q_sb[HD:P, 0:QPRI], 0.0)
        # v (dense, pre-transpose) lives on partitions 64:128 (the fused
        # k|v projection's PSUM rows), ready for the hardware transpose.
        v_sb = qpool.tile([P, S], BF16)
        # v^T blocks [j-part, (block, 128)]: cols 0:64 = v^T (the hardware
        # transpose requires the 128-stride block layout), col 64 = ones.
        # The AV stationary slices only cols 0:65.
        va_sb = qpool.tile([P, NJ * P], BF16)
        va_v = va_sb.rearrange("p (j c) -> p j c", c=P)
        nc.vector.memset(va_v[:, :, HD:HD + 1], 1.0)

        # ---- projections ----
        def kv_proj(s):
            # fused: stationary (wk^T | wv^T) -> PSUM rows 0:64 = k,
            # rows 64:128 = v, one matmul pass per 512-column slice
            sl = slice(s * 512, (s + 1) * 512)
            kvp = pp.tile([P, 512], F32, tag="av", bufs=2, name="kvp")
            nc.tensor.matmul(kvp[:], wkv_sb[0], xkv_v[:, 0, sl],
                             start=True, stop=False)
            nc.tensor.matmul(kvp[:], wkv_sb[1], xkv_v[:, 1, sl],
                             start=False, stop=True)
            nc.vector.tensor_copy(k_sb[0:HD, sl], kvp[0:HD, :])
            nc.vector.tensor_copy(v_sb[HD:P, sl], kvp[HD:P, :])

        def q_proj(t):
            sl = slice(t * 512, (t + 1) * 512)
            qp = pp.tile([HD, 512], F32, tag="av", bufs=2, name="qp")
            nc.tensor.matmul(qp[:], wq_sb[0], xq_v[:, 0, sl],
                             start=True, stop=False)
            nc.tensor.matmul(qp[:], wq_sb[1], xq_v[:, 1, sl],
                             start=False, stop=True)
            nc.vector.tensor_scalar_add(q_sb[0:HD, sl], qp[:], bq_sb[:])

        kv_proj(0)
        q_proj(0)
        q_proj(1)
        # deferred zero-pad remainders: DVE runs these ~16-22us while the
        # exp stream is already going; k columns 512:640 (j-block 4) are
        # needed first, at ~21us
        nc.vector.memset(k_sb[HD:P, KPRI:S], 0.0)
        nc.vector.memset(q_sb[HD:P, QPRI:S], 0.0)

        def transpose_piece(g):
            # v^T for j-blocks 8g..8g+7, available as soon as v slices
            # 2g/2g+1 are drained — the first AV only needs piece 0.
            nc.sync.dma_start_transpose(
                out=va_v[:, 8 * g:8 * (g + 1), 0:HD],
                in_=v_sb[HD:P, 1024 * g:1024 * (g + 1)])

        # ---- attention ----
        def qk_exp(c, j):
            st = pp.tile([P, IC], F32, tag="st", bufs=2, name="st")
            for h in range(IC // 512):
                isl = slice(c * IC + h * 512, c * IC + (h + 1) * 512)
                nc.tensor.matmul(st[:, h * 512:(h + 1) * 512],
                                 k_sb[:, j * P:(j + 1) * P],
                                 q_sb[:, isl],
                                 start=True, stop=True)
            et = epool.tile([P, IC], BF16, name="et")
            nc.scalar.activation(et[:], st[:],
                                 mybir.ActivationFunctionType.Exp,
                                 bias=zbias_sb[:])
            return et

        pend = [None] * NI

        def epilogue_part2(i, final=False, ohs=(0, 1)):
            outt = pend[i]
            for oh in ohs:
                for h in range(IC // 512):
                    yp = pp.tile([P, 512], F32, tag="av", bufs=2, name="yp")
                    nc.tensor.matmul(yp[:], wo_sb[:, oh * P:(oh + 1) * P],
                                     outt[:, h * 512:(h + 1) * 512],
                                     start=True, stop=True)
                    ys = fpool.tile([P, 512], BF16, name="ys")
                    if final and (oh + h) % 2 == 1:
                        nc.scalar.activation(
                            ys[:], yp[:], mybir.ActivationFunctionType.Copy)
                    else:
                        nc.vector.tensor_copy(ys[:], yp[:])
                    eng = nc.sync if oh == 0 else nc.scalar
                    eng.dma_start(
                        y[oh][:, i * IC + h * 512:i * IC + (h + 1) * 512],
                        ys[:])

        # Chunk-0 weave: remaining projections + transpose pieces ride the
        # exp-paced stream at at most one matmul-pair per j.
        weave0 = {
            1: [lambda: kv_proj(1)],
            2: [lambda: transpose_piece(0)],
            3: [lambda: kv_proj(2)],
            5: [lambda: kv_proj(3)],
            6: [lambda: transpose_piece(1)],
            7: [lambda: kv_proj(4)],
            11: [lambda: kv_proj(5)],
            12: [lambda: transpose_piece(2)],
            13: [lambda: kv_proj(6)],
            15: [lambda: kv_proj(7)],
            16: [lambda: transpose_piece(3)],
            17: [lambda: q_proj(2)],
            19: [lambda: q_proj(3)],
            21: [lambda: q_proj(4)],
            23: [lambda: q_proj(5)],
            25: [lambda: q_proj(6)],
            27: [lambda: q_proj(7)],
        }

        bank = []
        for j in range(PRE):
            bank.append(qk_exp(0, j))
            for fn in weave0.get(j, []):
                fn()

        for i in range(NI):
            av = pp.tile([HD + 1, IC], F32, tag="av", bufs=2, name="av")
            for j in range(NJ):
                if i > 0 and j == 8:
                    epilogue_part2(i - 1, ohs=(0,))
                if i > 0 and j == 10:
                    epilogue_part2(i - 1, ohs=(1,))
                if i == 0 and j < PRE:
                    et = bank[j]
                else:
                    et = qk_exp(i, j)
                    if i == 0:
                        for fn in weave0.get(j, []):
                            fn()
                for h in (1, 0):
                    nc.tensor.matmul(av[:, h * 512:(h + 1) * 512],
                                     va_v[:, j, 0:HD + 1],
                                     et[:, h * 512:(h + 1) * 512],
                                     start=(j == 0), stop=(j == NJ - 1))

            outt = fpool.tile([HD, IC], BF16, name="outt")
            if i == NI - 1:
                nc.vector.tensor_copy(outt[:, 0:512], av[0:HD, 0:512])
                nc.vector.tensor_copy(outt[:, 512:IC], av[0:HD, 512:IC])
            else:
                nc.vector.tensor_copy(outt[:], av[0:HD, :])
            den = fpool.tile([1, IC], F32, name="den")
            nc.vector.tensor_copy(den[:], av[HD:HD + 1, :])
            nc.gpsimd.dma_start(yden[:, i * IC:(i + 1) * IC], den[:])
            pend[i] = outt

        epilogue_part2(NI - 1, final=True)


def build():
    nc = bacc.Bacc("TRN2", target_bir_lowering=False, debug=False,
                   enable_asserts=False)
    with tile.TileContext(nc) as tc:
        _emit(tc)
    nc.compile()
    return nc


_NC_CACHE = []


def _get_nc():
    if not _NC_CACHE:
        _NC_CACHE.append(build())
    return _NC_CACHE[0]


def make_in_maps(x_q, x_kv, wq, bq, wk, bk, wv, bv, wo, bo):
    bf = ml_dtypes.bfloat16
    in_maps = []
    bo_effs = []
    for c in range(NCORES):
        b, n = divmod(c, NH)
        hs = slice(n * HD, (n + 1) * HD)
        wq_h = wq[hs].astype(np.float64) * SCALE
        bo_eff = wo[:, hs].astype(np.float64) @ bv[hs].astype(np.float64)
        if n == 0:
            bo_eff = bo_eff + bo.astype(np.float64)
        bo_effs.append(bo_eff.astype(np.float32))
        in_maps.append({
            "xq": np.ascontiguousarray(
                x_q[b].reshape(C, S).reshape(2, P, S)).astype(bf),
            "xkv": np.ascontiguousarray(
                x_kv[b].reshape(C, S).reshape(2, P, S)).astype(bf),
            "wblob": np.ascontiguousarray(
                np.concatenate(
                    [np.concatenate([wk[hs].T, wv[hs].T], axis=1)
                     .reshape(2, P, P),
                     wq_h.T.reshape(2, P, HD)], axis=2)).astype(bf),
            "woT": np.ascontiguousarray(wo[:, hs].T).astype(bf),
            "bq": (bq[hs].astype(np.float64) * SCALE
                   ).astype(np.float32).reshape(HD, 1),
        })
    return in_maps, bo_effs


def assemble_output(results, bo_effs):
    # y_core is the unnormalized head partial; divide by the softmax
    # denominator and add the (host-folded) bias here.
    y = np.zeros((B, C, S), np.float32)
    for c in range(NCORES):
        b = c // NH
        den = results[c]["yden"].reshape(1, S)
        y[b] += results[c]["y"].astype(np.float32).reshape(C, S) / den \
            + bo_effs[c].reshape(C, 1)
    return y.reshape(B, C, HGT, WID)


def kernel(**inputs):
    nc = _get_nc()
    in_maps, bo_effs = make_in_maps(**inputs)
    res = run_bass_kernel_spmd(nc, in_maps, list(range(NCORES)))
    return assemble_output(res.results, bo_effs)


if __name__ == "__main__":
    nc = build()
    print("built + compiled ok")


# revision 39
# speedup vs baseline: 1.1661x; 1.0126x over previous
"""Cross-attention kernel for Trainium2, sharded over 8 NeuronCores.

Problem (per reference):
  q = wq @ x_q + bq ; k = wk @ x_kv + bk ; v = wv @ x_kv + bv   (1x1 convs)
  per head: attn = softmax(q^T k / sqrt(hd)) ; out = attn @ v^T
  y = wo @ out + bo

Sharding: core c -> (batch b = c // 4, head n = c % 4). Each core runs one
head's full attention and produces the partial output projection
y_part = wo[:, head] @ out_head; the host sums the 4 head partials per batch.

Mathematically exact simplifications (as in the 192us version):
  * bk drops (per-query logit shift cancels in softmax); bv folds into the
    host-side bias (softmax rows sum to 1); scale folds into wq/bq;
    no max-subtraction (logits ~N(0,1)); softmax denominator comes from a
    ones-column in the AV stationary; normalization deferred to the host
    (ships y_un + per-pixel denominators, host divides).

Speed structure. The scalar engine's exp stream is the pacer: 128 exp
instructions of [128,1024] at ~1.11us each (1 elem/cycle/partition at
1.2GHz + ~230ns instruction overhead) ~= 142us; everything else must hide
under it. Measured facts this schedule is built on (NTFF traces):
  * A 512-col matmul costs ~215ns streaming at the promoted 2.4GHz PE
    clock + ~100-135ns weight load; per attention iteration the PE runs
    QK(2) + AV(2) matmuls ~= 1.05-1.25us, just under the exp. fp8
    DoubleRow gives NO streaming speedup on this silicon (tried: 512-out
    DoubleRow measures ~375ns like bf16), so everything stays bf16.
  * The PE p-state: ~5us of continuous warmup matmuls promote 1.2->2.4GHz
    (~13us in, right before the first projection); the exp-paced stream's
    small per-iteration gaps then hold it.
  * Each early dma_start issued on the scalar ring costs ~3.5us of ACT
    sequencer time, so the scalar ring carries only late y-output DMAs;
    inputs ride the sync ring (priority slices first: first 512 xkv cols,
    first 1024 xq cols, weights at the head) and the gpsimd SWDGE queue.
  * The remaining k|v / q projections are woven into the chunk-0 stream at
    at most one matmul-pair per j (PE executes in order; heavier weaves
    starve the exp stream 1:1, lighter ones let the p-state drop).
  * v^T is produced per-1024-column piece: fused k|v projection, DVE
    drain, and four piece-granular hardware DMA transposes — the first AV
    only waits for piece 0, so only PRE=4 exps need banking and the
    end-of-stream AV debt (the kernel tail) stays small.
  * The AV stationary is trimmed to 65 columns (64 v^T + ones).
"""

import numpy as np
import ml_dtypes

import concourse.bacc as bacc
import concourse.mybir as mybir
import concourse.tile as tile
from concourse.bass_utils import run_bass_kernel_spmd

F32 = mybir.dt.float32
BF16 = mybir.dt.bfloat16

B, C, HGT, WID = 2, 256, 64, 64
S = HGT * WID  # 4096 pixels
NH, HD = 4, 64
NCORES = 8
P = 128
IC = 1024  # i-chunk width (2 PSUM banks)
NI = S // IC  # 4
NJ = S // P  # 32 j-blocks
SCALE = HD ** -0.5
KPRI = 512   # priority xkv columns (first k|v projection slice)
QPRI = 1024  # priority xq columns (chunk-0 q projection)
PRE = 6      # chunk-0 exps banked ahead of the first AV (covers the
             # first v-transpose piece)


def _emit(tc):
    nc = tc.nc
    xq = nc.dram_tensor("xq", [2, P, S], BF16, kind="ExternalInput").ap()
    xkv = nc.dram_tensor("xkv", [2, P, S], BF16, kind="ExternalInput").ap()
    # wblob packs (wk|wv) [128 cols] + wq [64 cols] per channel half: the
    # critical-path weights arrive in ONE transfer (each DMA piece carries
    # ~1.3us of fixed cost on the ring — small pieces starve the prologue)
    wblob = nc.dram_tensor("wblob", [2, P, P + HD], BF16,
                           kind="ExternalInput").ap()
    woT = nc.dram_tensor("woT", [HD, C], BF16, kind="ExternalInput").ap()
    bq = nc.dram_tensor("bq", [HD, 1], F32, kind="ExternalInput").ap()
    y = nc.dram_tensor("y", [2, P, S], BF16, kind="ExternalOutput").ap()
    yden = nc.dram_tensor("yden", [1, S], F32, kind="ExternalOutput").ap()

    with (
        tc.tile_pool(name="const", bufs=1) as cpool,
        tc.tile_pool(name="xp", bufs=1) as xpool,
        tc.tile_pool(name="qkv", bufs=1) as qpool,
        tc.tile_pool(name="es", bufs=10) as epool,
        tc.tile_pool(name="epi", bufs=2) as fpool,
        tc.tile_pool(name="ps", bufs=2, space="PSUM") as pp,
    ):
        # ---- critical-path weights + priority slices head the sync ring,
        # consolidated into FEW LARGE transfers (per-piece fixed cost
        # ~1.3us; 12 small pieces pushed the priority data past the
        # warmup's end and let the PE p-state demote) ----
        wb_sb = cpool.tile([P, 2 * (P + HD)], BF16)
        wb_v = wb_sb.rearrange("p (c w) -> p c w", c=2)
        wkv_sb = [wb_v[:, cch, 0:P] for cch in range(2)]
        wq_sb = [wb_v[:, cch, P:P + HD] for cch in range(2)]
        bq_sb = cpool.tile([HD, 1], F32)
        # wblob rides the scalar ring: its one dma_start costs ~3.5us of
        # ACT sequencer at boot, when ACT is idle anyway, and keeps the
        # sync ring's head free for the xq priority half.
        nc.scalar.dma_start(wb_v[:, :, :], wblob.rearrange("c p w -> p c w"))

        # activations as single tiles [p, (half, col)] so one DMA can carry
        # both channel halves of a column range; DRAM-side APs are
        # rearranged to (p, half, col) so source and dest walk in the same
        # dimension order
        xq_sb = xpool.tile([P, 2 * S], BF16)
        xq_v = xq_sb.rearrange("p (c w) -> p c w", c=2)
        xkv_sb = xpool.tile([P, 2 * S], BF16)
        xkv_v = xkv_sb.rearrange("p (c w) -> p c w", c=2)
        xq_r = xq.rearrange("c p w -> p c w")
        xkv_r = xkv.rearrange("c p w -> p c w")
        # Inputs ride sync + SWDGE only: early dma_starts on the scalar
        # ring cost ~3.5us of ACT sequencer each, straight off the pacer.
        # The first-exp chain splits across the two queues: sync carries
        # xq half 0, SWDGE carries xq half 1 + the xkv priority columns.
        nc.sync.dma_start(xkv_v[:, :, 0:KPRI], xkv_r[:, :, 0:KPRI])
        nc.sync.dma_start(xq_v[:, 0, 0:QPRI], xq[0][:, 0:QPRI])
        nc.sync.dma_start(bq_sb[:], bq)
        nc.sync.dma_start(xkv_v[:, 0, 512:1536], xkv[0][:, 512:1536])
        nc.sync.dma_start(xkv_v[:, 0, 1536:2560], xkv[0][:, 1536:2560])

        # ---- the rest on the gpsimd SWDGE queue (self-issued) ----
        nc.gpsimd.dma_start(xq_v[:, 1, 0:QPRI], xq[1][:, 0:QPRI])
        nc.gpsimd.dma_start(xkv_v[:, 1, 512:1536], xkv[1][:, 512:1536])
        wo_sb = cpool.tile([HD, C], BF16)
        nc.gpsimd.dma_start(wo_sb[:], woT)
        nc.gpsimd.dma_start(xkv_v[:, 1, 1536:2560], xkv[1][:, 1536:2560])
        nc.gpsimd.dma_start(xkv_v[:, 1, 2560:S], xkv[1][:, 2560:S])
        nc.gpsimd.dma_start(xkv_v[:, 0, 2560:S], xkv[0][:, 2560:S])
        for t in range(1, S // 1024):
            sl = slice(t * 1024, (t + 1) * 1024)
            nc.gpsimd.dma_start(xq_v[:, :, sl], xq_r[:, :, sl])

        # Zero bias for exp via memset (a float bias would become a DMA'd
        # const tensor queued behind the input DMAs).
        zbias_sb = cpool.tile([P, 1], F32)
        nc.vector.memset(zbias_sb[:], 0.0)

        # PE warmup burst: ~10us of dense matmuls while the input DMAs are
        # in flight; the activity monitor promotes the PE to 2.4GHz after
        # ~5us of sustained streaming, right before the first projection.
        wrm_sb = cpool.tile([P, 512], BF16)
        nc.vector.memset(wrm_sb[:], 0.0)
        for w in range(12):
            wp = pp.tile([P, 512], F32, tag="st", bufs=2, name="wp")
            nc.tensor.matmul(wp[:], wrm_sb[:, 0:P], wrm_sb[:],
                             start=True, stop=True)
        # Warmup exp so the ~2.7us activation-table load happens before the
        # first real exp.
        warm_sb = cpool.tile([P, 1], BF16)
        nc.scalar.activation(warm_sb[:], zbias_sb[:],
                             mybir.ActivationFunctionType.Exp,
                             bias=zbias_sb[:])

        # q/k zero-padded to 128 partitions (the zero rows contribute
        # nothing to the contraction). Only the slices the first exps need
        # are zeroed up front: a full-width [64,4096] DVE memset costs
        # ~3.5us and would queue ahead of the projection drains, gating the
        # first exp; the rest is deferred until the exp stream is running.
        q_sb = qpool.tile([P, S], BF16)
        k_sb = qpool.tile([P, S], BF16)
        nc.vector.memset(k_sb[HD:P, 0:KPRI], 0.0)
        nc.vector.memset(q_sb[HD:P, 0:QPRI], 0.0)
        # v (dense, pre-transpose) lives on partitions 64:128 (the fused
        # k|v projection's PSUM rows), ready for the hardware transpose.
        v_sb = qpool.tile([P, S], BF16)
        # v^T blocks [j-part, (block, 128)]: cols 0:64 = v^T (the hardware
        # transpose requires the 128-stride block layout), col 64 = ones.
        # The AV stationary slices only cols 0:65.
        va_sb = qpool.tile([P, NJ * P], BF16)
        va_v = va_sb.rearrange("p (j c) -> p j c", c=P)
        nc.vector.memset(va_v[:, :, HD:HD + 1], 1.0)

        # ---- projections ----
        def kv_proj(s):
            # fused: stationary (wk^T | wv^T) -> PSUM rows 0:64 = k,
            # rows 64:128 = v, one matmul pass per 512-column slice
            sl = slice(s * 512, (s + 1) * 512)
            kvp = pp.tile([P, 512], F32, tag="av", bufs=2, name="kvp")
            nc.tensor.matmul(kvp[:], wkv_sb[0], xkv_v[:, 0, sl],
                             start=True, stop=False)
            nc.tensor.matmul(kvp[:], wkv_sb[1], xkv_v[:, 1, sl],
                             start=False, stop=True)
            nc.vector.tensor_copy(k_sb[0:HD, sl], kvp[0:HD, :])
            nc.vector.tensor_copy(v_sb[HD:P, sl], kvp[HD:P, :])

        def q_proj(t):
            sl = slice(t * 512, (t + 1) * 512)
            qp = pp.tile([HD, 512], F32, tag="av", bufs=2, name="qp")
            nc.tensor.matmul(qp[:], wq_sb[0], xq_v[:, 0, sl],
                             start=True, stop=False)
            nc.tensor.matmul(qp[:], wq_sb[1], xq_v[:, 1, sl],
                             start=False, stop=True)
            nc.vector.tensor_scalar_add(q_sb[0:HD, sl], qp[:], bq_sb[:])

        kv_proj(0)
        q_proj(0)
        q_proj(1)
        # deferred zero-pad remainders: DVE runs these ~16-22us while the
        # exp stream is already going; k columns 512:640 (j-block 4) are
        # needed first, at ~21us
        nc.vector.memset(k_sb[HD:P, KPRI:S], 0.0)
        nc.vector.memset(q_sb[HD:P, QPRI:S], 0.0)

        def transpose_piece(g):
            # v^T for j-blocks 8g..8g+7, available as soon as v slices
            # 2g/2g+1 are drained — the first AV only needs piece 0.
            nc.sync.dma_start_transpose(
                out=va_v[:, 8 * g:8 * (g + 1), 0:HD],
                in_=v_sb[HD:P, 1024 * g:1024 * (g + 1)])

        # ---- attention ----
        def qk_exp(c, j):
            st = pp.tile([P, IC], F32, tag="st", bufs=2, name="st")
            for h in range(IC // 512):
                isl = slice(c * IC + h * 512, c * IC + (h + 1) * 512)
                nc.tensor.matmul(st[:, h * 512:(h + 1) * 512],
                                 k_sb[:, j * P:(j + 1) * P],
                                 q_sb[:, isl],
                                 start=True, stop=True)
            et = epool.tile([P, IC], BF16, name="et")
            nc.scalar.activation(et[:], st[:],
                                 mybir.ActivationFunctionType.Exp,
                                 bias=zbias_sb[:])
            return et

        pend = [None] * NI

        def epilogue_part2(i, final=False, ohs=(0, 1)):
            outt = pend[i]
            for oh in ohs:
                for h in range(IC // 512):
                    yp = pp.tile([P, 512], F32, tag="av", bufs=2, name="yp")
                    nc.tensor.matmul(yp[:], wo_sb[:, oh * P:(oh + 1) * P],
                                     outt[:, h * 512:(h + 1) * 512],
                                     start=True, stop=True)
                    ys = fpool.tile([P, 512], BF16, name="ys")
                    if final and (oh + h) % 2 == 1:
                        nc.scalar.activation(
                            ys[:], yp[:], mybir.ActivationFunctionType.Copy)
                    else:
                        nc.vector.tensor_copy(ys[:], yp[:])
                    eng = nc.sync if oh == 0 else nc.scalar
                    eng.dma_start(
                        y[oh][:, i * IC + h * 512:i * IC + (h + 1) * 512],
                        ys[:])

        # Chunk-0 weave: remaining projections + transpose pieces ride the
        # exp-paced stream at at most one matmul-pair per j.
        weave0 = {
            1: [lambda: kv_proj(1)],
            2: [lambda: transpose_piece(0)],
            3: [lambda: kv_proj(2)],
            5: [lambda: kv_proj(3)],
            6: [lambda: transpose_piece(1)],
            7: [lambda: kv_proj(4)],
            11: [lambda: kv_proj(5)],
            12: [lambda: transpose_piece(2)],
            13: [lambda: kv_proj(6)],
            15: [lambda: kv_proj(7)],
            16: [lambda: transpose_piece(3)],
            17: [lambda: q_proj(2)],
            19: [lambda: q_proj(3)],
            21: [lambda: q_proj(4)],
            23: [lambda: q_proj(5)],
            25: [lambda: q_proj(6)],
            27: [lambda: q_proj(7)],
        }

        bank = []
        for j in range(PRE):
            bank.append(qk_exp(0, j))
            for fn in weave0.get(j, []):
                fn()

        for i in range(NI):
            av = pp.tile([HD + 1, IC], F32, tag="av", bufs=2, name="av")
            for j in range(NJ):
                if i > 0 and j == 8:
                    epilogue_part2(i - 1, ohs=(0,))
                if i > 0 and j == 10:
                    epilogue_part2(i - 1, ohs=(1,))
                if i == 0 and j < PRE:
                    et = bank[j]
                else:
                    et = qk_exp(i, j)
                    if i == 0:
                        for fn in weave0.get(j, []):
                            fn()
                for h in (1, 0):
                    nc.tensor.matmul(av[:, h * 512:(h + 1) * 512],
                                     va_v[:, j, 0:HD + 1],
                                     et[:, h * 512:(h + 1) * 512],
                                     start=(j == 0), stop=(j == NJ - 1))

            outt = fpool.tile([HD, IC], BF16, name="outt")
            if i == NI - 1:
                nc.vector.tensor_copy(outt[:, 0:512], av[0:HD, 0:512])
                nc.vector.tensor_copy(outt[:, 512:IC], av[0:HD, 512:IC])
            else:
                nc.vector.tensor_copy(outt[:], av[0:HD, :])
            den = fpool.tile([1, IC], F32, name="den")
            nc.vector.tensor_copy(den[:], av[HD:HD + 1, :])
            nc.gpsimd.dma_start(yden[:, i * IC:(i + 1) * IC], den[:])
            pend[i] = outt

        epilogue_part2(NI - 1, final=True)


def build():
    nc = bacc.Bacc("TRN2", target_bir_lowering=False, debug=False,
                   enable_asserts=False)
    with tile.TileContext(nc) as tc:
        _emit(tc)
    nc.compile()
    return nc


_NC_CACHE = []


def _get_nc():
    if not _NC_CACHE:
        _NC_CACHE.append(build())
    return _NC_CACHE[0]


def make_in_maps(x_q, x_kv, wq, bq, wk, bk, wv, bv, wo, bo):
    bf = ml_dtypes.bfloat16
    in_maps = []
    bo_effs = []
    for c in range(NCORES):
        b, n = divmod(c, NH)
        hs = slice(n * HD, (n + 1) * HD)
        wq_h = wq[hs].astype(np.float64) * SCALE
        bo_eff = wo[:, hs].astype(np.float64) @ bv[hs].astype(np.float64)
        if n == 0:
            bo_eff = bo_eff + bo.astype(np.float64)
        bo_effs.append(bo_eff.astype(np.float32))
        in_maps.append({
            "xq": np.ascontiguousarray(
                x_q[b].reshape(C, S).reshape(2, P, S)).astype(bf),
            "xkv": np.ascontiguousarray(
                x_kv[b].reshape(C, S).reshape(2, P, S)).astype(bf),
            "wblob": np.ascontiguousarray(
                np.concatenate(
                    [np.concatenate([wk[hs].T, wv[hs].T], axis=1)
                     .reshape(2, P, P),
                     wq_h.T.reshape(2, P, HD)], axis=2)).astype(bf),
            "woT": np.ascontiguousarray(wo[:, hs].T).astype(bf),
            "bq": (bq[hs].astype(np.float64) * SCALE
                   ).astype(np.float32).reshape(HD, 1),
        })
    return in_maps, bo_effs


def assemble_output(results, bo_effs):
    # y_core is the unnormalized head partial; divide by the softmax
    # denominator and add the (host-folded) bias here.
    y = np.zeros((B, C, S), np.float32)
    for c in range(NCORES):
        b = c // NH
        den = results[c]["yden"].reshape(1, S)
        y[b] += results[c]["y"].astype(np.float32).reshape(C, S) / den \
            + bo_effs[c].reshape(C, 1)
    return y.reshape(B, C, HGT, WID)


def kernel(**inputs):
    nc = _get_nc()
    in_maps, bo_effs = make_in_maps(**inputs)
    res = run_bass_kernel_spmd(nc, in_maps, list(range(NCORES)))
    return assemble_output(res.results, bo_effs)


if __name__ == "__main__":
    nc = build()
    print("built + compiled ok")


# revision 41
# speedup vs baseline: 1.1682x; 1.0018x over previous
"""Cross-attention kernel for Trainium2, sharded over 8 NeuronCores.

Problem (per reference):
  q = wq @ x_q + bq ; k = wk @ x_kv + bk ; v = wv @ x_kv + bv   (1x1 convs)
  per head: attn = softmax(q^T k / sqrt(hd)) ; out = attn @ v^T
  y = wo @ out + bo

Sharding: core c -> (batch b = c // 4, head n = c % 4). Each core runs one
head's full attention and produces the partial output projection
y_part = wo[:, head] @ out_head; the host sums the 4 head partials per batch.

Mathematically exact simplifications (as in the 192us version):
  * bk drops (per-query logit shift cancels in softmax); bv folds into the
    host-side bias (softmax rows sum to 1); scale folds into wq/bq;
    no max-subtraction (logits ~N(0,1)); softmax denominator comes from a
    ones-column in the AV stationary; normalization deferred to the host
    (ships y_un + per-pixel denominators, host divides).

Speed structure. The scalar engine's exp stream is the pacer: 128 exp
instructions of [128,1024] at ~1.11us each (1 elem/cycle/partition at
1.2GHz + ~230ns instruction overhead) ~= 142us; everything else must hide
under it. Measured facts this schedule is built on (NTFF traces):
  * A 512-col matmul costs ~215ns streaming at the promoted 2.4GHz PE
    clock + ~100-135ns weight load; per attention iteration the PE runs
    QK(2) + AV(2) matmuls ~= 1.05-1.25us, just under the exp. fp8
    DoubleRow gives NO streaming speedup on this silicon (tried: 512-out
    DoubleRow measures ~375ns like bf16), so everything stays bf16.
  * The PE p-state: ~5us of continuous warmup matmuls promote 1.2->2.4GHz
    (~13us in, right before the first projection); the exp-paced stream's
    small per-iteration gaps then hold it.
  * Each early dma_start issued on the scalar ring costs ~3.5us of ACT
    sequencer time, so the scalar ring carries only late y-output DMAs;
    inputs ride the sync ring (priority slices first: first 512 xkv cols,
    first 1024 xq cols, weights at the head) and the gpsimd SWDGE queue.
  * The remaining k|v / q projections are woven into the chunk-0 stream at
    at most one matmul-pair per j (PE executes in order; heavier weaves
    starve the exp stream 1:1, lighter ones let the p-state drop).
  * v^T is produced per-1024-column piece: fused k|v projection, DVE
    drain, and four piece-granular hardware DMA transposes — the first AV
    only waits for piece 0, so only PRE=4 exps need banking and the
    end-of-stream AV debt (the kernel tail) stays small.
  * The AV stationary is trimmed to 65 columns (64 v^T + ones).
"""

import numpy as np
import ml_dtypes

import concourse.bacc as bacc
import concourse.mybir as mybir
import concourse.tile as tile
from concourse.bass_utils import run_bass_kernel_spmd

F32 = mybir.dt.float32
BF16 = mybir.dt.bfloat16

B, C, HGT, WID = 2, 256, 64, 64
S = HGT * WID  # 4096 pixels
NH, HD = 4, 64
NCORES = 8
P = 128
IC = 1024  # i-chunk width (2 PSUM banks)
NI = S // IC  # 4
NJ = S // P  # 32 j-blocks
SCALE = HD ** -0.5
KPRI = 512   # priority xkv columns (first k|v projection slice)
QPRI = 1024  # priority xq columns (chunk-0 q projection)
PRE = 7      # chunk-0 exps banked ahead of the first AV (covers the
             # first v-transpose piece)


def _emit(tc):
    nc = tc.nc
    xq = nc.dram_tensor("xq", [2, P, S], BF16, kind="ExternalInput").ap()
    xkv = nc.dram_tensor("xkv", [2, P, S], BF16, kind="ExternalInput").ap()
    # wblob packs (wk|wv) [128 cols] + wq [64 cols] per channel half: the
    # critical-path weights arrive in ONE transfer (each DMA piece carries
    # ~1.3us of fixed cost on the ring — small pieces starve the prologue)
    wblob = nc.dram_tensor("wblob", [2, P, P + HD], BF16,
                           kind="ExternalInput").ap()
    woT = nc.dram_tensor("woT", [HD, C], BF16, kind="ExternalInput").ap()
    bq = nc.dram_tensor("bq", [HD, 1], F32, kind="ExternalInput").ap()
    y = nc.dram_tensor("y", [2, P, S], BF16, kind="ExternalOutput").ap()
    yden = nc.dram_tensor("yden", [1, S], F32, kind="ExternalOutput").ap()

    with (
        tc.tile_pool(name="const", bufs=1) as cpool,
        tc.tile_pool(name="xp", bufs=1) as xpool,
        tc.tile_pool(name="qkv", bufs=1) as qpool,
        tc.tile_pool(name="es", bufs=10) as epool,
        tc.tile_pool(name="epi", bufs=2) as fpool,
        tc.tile_pool(name="ps", bufs=2, space="PSUM") as pp,
    ):
        # ---- critical-path weights + priority slices head the sync ring,
        # consolidated into FEW LARGE transfers (per-piece fixed cost
        # ~1.3us; 12 small pieces pushed the priority data past the
        # warmup's end and let the PE p-state demote) ----
        wb_sb = cpool.tile([P, 2 * (P + HD)], BF16)
        wb_v = wb_sb.rearrange("p (c w) -> p c w", c=2)
        wkv_sb = [wb_v[:, cch, 0:P] for cch in range(2)]
        wq_sb = [wb_v[:, cch, P:P + HD] for cch in range(2)]
        bq_sb = cpool.tile([HD, 1], F32)
        # wblob rides the scalar ring: its one dma_start costs ~3.5us of
        # ACT sequencer at boot, when ACT is idle anyway, and keeps the
        # sync ring's head free for the xq priority half.
        nc.scalar.dma_start(wb_v[:, :, :], wblob.rearrange("c p w -> p c w"))

        # activations as single tiles [p, (half, col)] so one DMA can carry
        # both channel halves of a column range; DRAM-side APs are
        # rearranged to (p, half, col) so source and dest walk in the same
        # dimension order
        xq_sb = xpool.tile([P, 2 * S], BF16)
        xq_v = xq_sb.rearrange("p (c w) -> p c w", c=2)
        xkv_sb = xpool.tile([P, 2 * S], BF16)
        xkv_v = xkv_sb.rearrange("p (c w) -> p c w", c=2)
        xq_r = xq.rearrange("c p w -> p c w")
        xkv_r = xkv.rearrange("c p w -> p c w")
        # Inputs ride sync + SWDGE only: early dma_starts on the scalar
        # ring cost ~3.5us of ACT sequencer each, straight off the pacer.
        # The first-exp chain splits across the two queues: sync carries
        # xq half 0, SWDGE carries xq half 1 + the xkv priority columns.
        nc.sync.dma_start(xkv_v[:, :, 0:KPRI], xkv_r[:, :, 0:KPRI])
        nc.sync.dma_start(xq_v[:, 0, 0:QPRI], xq[0][:, 0:QPRI])
        nc.sync.dma_start(bq_sb[:], bq)
        nc.sync.dma_start(xkv_v[:, 0, 512:1536], xkv[0][:, 512:1536])
        nc.sync.dma_start(xkv_v[:, 0, 1536:2560], xkv[0][:, 1536:2560])

        # ---- the rest on the gpsimd SWDGE queue (self-issued) ----
        nc.gpsimd.dma_start(xq_v[:, 1, 0:QPRI], xq[1][:, 0:QPRI])
        nc.gpsimd.dma_start(xkv_v[:, 1, 512:1536], xkv[1][:, 512:1536])
        wo_sb = cpool.tile([HD, C], BF16)
        nc.gpsimd.dma_start(wo_sb[:], woT)
        nc.gpsimd.dma_start(xkv_v[:, 1, 1536:2560], xkv[1][:, 1536:2560])
        nc.gpsimd.dma_start(xkv_v[:, 1, 2560:S], xkv[1][:, 2560:S])
        nc.gpsimd.dma_start(xkv_v[:, 0, 2560:S], xkv[0][:, 2560:S])
        for t in range(1, S // 1024):
            sl = slice(t * 1024, (t + 1) * 1024)
            nc.gpsimd.dma_start(xq_v[:, :, sl], xq_r[:, :, sl])

        # Zero bias for exp via memset (a float bias would become a DMA'd
        # const tensor queued behind the input DMAs).
        zbias_sb = cpool.tile([P, 1], F32)
        nc.vector.memset(zbias_sb[:], 0.0)

        # PE warmup burst: ~10us of dense matmuls while the input DMAs are
        # in flight; the activity monitor promotes the PE to 2.4GHz after
        # ~5us of sustained streaming, right before the first projection.
        wrm_sb = cpool.tile([P, 512], BF16)
        nc.vector.memset(wrm_sb[:], 0.0)
        for w in range(12):
            wp = pp.tile([P, 512], F32, tag="st", bufs=2, name="wp")
            nc.tensor.matmul(wp[:], wrm_sb[:, 0:P], wrm_sb[:],
                             start=True, stop=True)
        # Warmup exp so the ~2.7us activation-table load happens before the
        # first real exp.
        warm_sb = cpool.tile([P, 1], BF16)
        nc.scalar.activation(warm_sb[:], zbias_sb[:],
                             mybir.ActivationFunctionType.Exp,
                             bias=zbias_sb[:])

        # q/k zero-padded to 128 partitions (the zero rows contribute
        # nothing to the contraction). Only the slices the first exps need
        # are zeroed up front: a full-width [64,4096] DVE memset costs
        # ~3.5us and would queue ahead of the projection drains, gating the
        # first exp; the rest is deferred until the exp stream is running.
        q_sb = qpool.tile([P, S], BF16)
        k_sb = qpool.tile([P, S], BF16)
        nc.vector.memset(k_sb[HD:P, 0:KPRI], 0.0)
        nc.vector.memset(q_sb[HD:P, 0:QPRI], 0.0)
        # v (dense, pre-transpose) lives on partitions 64:128 (the fused
        # k|v projection's PSUM rows), ready for the hardware transpose.
        v_sb = qpool.tile([P, S], BF16)
        # v^T blocks [j-part, (block, 128)]: cols 0:64 = v^T (the hardware
        # transpose requires the 128-stride block layout), col 64 = ones.
        # The AV stationary slices only cols 0:65.
        va_sb = qpool.tile([P, NJ * P], BF16)
        va_v = va_sb.rearrange("p (j c) -> p j c", c=P)
        nc.vector.memset(va_v[:, :, HD:HD + 1], 1.0)

        # ---- projections ----
        def kv_proj(s):
            # fused: stationary (wk^T | wv^T) -> PSUM rows 0:64 = k,
            # rows 64:128 = v, one matmul pass per 512-column slice
            sl = slice(s * 512, (s + 1) * 512)
            kvp = pp.tile([P, 512], F32, tag="av", bufs=2, name="kvp")
            nc.tensor.matmul(kvp[:], wkv_sb[0], xkv_v[:, 0, sl],
                             start=True, stop=False)
            nc.tensor.matmul(kvp[:], wkv_sb[1], xkv_v[:, 1, sl],
                             start=False, stop=True)
            nc.vector.tensor_copy(k_sb[0:HD, sl], kvp[0:HD, :])
            nc.vector.tensor_copy(v_sb[HD:P, sl], kvp[HD:P, :])

        def q_proj(t):
            sl = slice(t * 512, (t + 1) * 512)
            qp = pp.tile([HD, 512], F32, tag="av", bufs=2, name="qp")
            nc.tensor.matmul(qp[:], wq_sb[0], xq_v[:, 0, sl],
                             start=True, stop=False)
            nc.tensor.matmul(qp[:], wq_sb[1], xq_v[:, 1, sl],
                             start=False, stop=True)
            nc.vector.tensor_scalar_add(q_sb[0:HD, sl], qp[:], bq_sb[:])

        kv_proj(0)
        q_proj(0)
        q_proj(1)
        # deferred zero-pad remainders: DVE runs these ~16-22us while the
        # exp stream is already going; k columns 512:640 (j-block 4) are
        # needed first, at ~21us
        nc.vector.memset(k_sb[HD:P, KPRI:S], 0.0)
        nc.vector.memset(q_sb[HD:P, QPRI:S], 0.0)

        def transpose_piece(g):
            # v^T for j-blocks 8g..8g+7, available as soon as v slices
            # 2g/2g+1 are drained — the first AV only needs piece 0.
            nc.sync.dma_start_transpose(
                out=va_v[:, 8 * g:8 * (g + 1), 0:HD],
                in_=v_sb[HD:P, 1024 * g:1024 * (g + 1)])

        # ---- attention ----
        def qk_exp(c, j):
            st = pp.tile([P, IC], F32, tag="st", bufs=2, name="st")
            for h in range(IC // 512):
                isl = slice(c * IC + h * 512, c * IC + (h + 1) * 512)
                nc.tensor.matmul(st[:, h * 512:(h + 1) * 512],
                                 k_sb[:, j * P:(j + 1) * P],
                                 q_sb[:, isl],
                                 start=True, stop=True)
            et = epool.tile([P, IC], BF16, name="et")
            nc.scalar.activation(et[:], st[:],
                                 mybir.ActivationFunctionType.Exp,
                                 bias=zbias_sb[:])
            return et

        pend = [None] * NI

        def epilogue_part2(i, final=False, ohs=(0, 1)):
            outt = pend[i]
            for oh in ohs:
                for h in range(IC // 512):
                    yp = pp.tile([P, 512], F32, tag="av", bufs=2, name="yp")
                    nc.tensor.matmul(yp[:], wo_sb[:, oh * P:(oh + 1) * P],
                                     outt[:, h * 512:(h + 1) * 512],
                                     start=True, stop=True)
                    ys = fpool.tile([P, 512], BF16, name="ys")
                    if final and (oh + h) % 2 == 1:
                        nc.scalar.activation(
                            ys[:], yp[:], mybir.ActivationFunctionType.Copy)
                    else:
                        nc.vector.tensor_copy(ys[:], yp[:])
                    eng = nc.sync if oh == 0 else nc.scalar
                    eng.dma_start(
                        y[oh][:, i * IC + h * 512:i * IC + (h + 1) * 512],
                        ys[:])

        # Chunk-0 weave: remaining projections + transpose pieces ride the
        # exp-paced stream at at most one matmul-pair per j.
        weave0 = {
            0: [lambda: kv_proj(1)],
            1: [lambda: transpose_piece(0)],
            2: [lambda: kv_proj(2)],
            4: [lambda: kv_proj(3)],
            5: [lambda: transpose_piece(1)],
            6: [lambda: kv_proj(4)],
            10: [lambda: kv_proj(5)],
            11: [lambda: transpose_piece(2)],
            12: [lambda: kv_proj(6)],
            14: [lambda: kv_proj(7)],
            15: [lambda: transpose_piece(3)],
            16: [lambda: q_proj(2)],
            18: [lambda: q_proj(3)],
            20: [lambda: q_proj(4)],
            22: [lambda: q_proj(5)],
            24: [lambda: q_proj(6)],
            26: [lambda: q_proj(7)],
        }

        bank = []
        for j in range(PRE):
            bank.append(qk_exp(0, j))
            for fn in weave0.get(j, []):
                fn()

        for i in range(NI):
            av = pp.tile([HD + 1, IC], F32, tag="av", bufs=2, name="av")
            for j in range(NJ):
                if i > 0 and j == 8:
                    epilogue_part2(i - 1, ohs=(0,))
                if i > 0 and j == 10:
                    epilogue_part2(i - 1, ohs=(1,))
                if i == 0 and j < PRE:
                    et = bank[j]
                else:
                    et = qk_exp(i, j)
                    if i == 0:
                        for fn in weave0.get(j, []):
                            fn()
                for h in (1, 0):
                    nc.tensor.matmul(av[:, h * 512:(h + 1) * 512],
                                     va_v[:, j, 0:HD + 1],
                                     et[:, h * 512:(h + 1) * 512],
                                     start=(j == 0), stop=(j == NJ - 1))

            outt = fpool.tile([HD, IC], BF16, name="outt")
            if i == NI - 1:
                nc.vector.tensor_copy(outt[:, 0:512], av[0:HD, 0:512])
                nc.vector.tensor_copy(outt[:, 512:IC], av[0:HD, 512:IC])
            else:
                nc.vector.tensor_copy(outt[:], av[0:HD, :])
            den = fpool.tile([1, IC], F32, name="den")
            nc.vector.tensor_copy(den[:], av[HD:HD + 1, :])
            nc.gpsimd.dma_start(yden[:, i * IC:(i + 1) * IC], den[:])
            pend[i] = outt

        epilogue_part2(NI - 1, final=True)


def build():
    nc = bacc.Bacc("TRN2", target_bir_lowering=False, debug=False,
                   enable_asserts=False)
    with tile.TileContext(nc) as tc:
        _emit(tc)
    nc.compile()
    return nc


_NC_CACHE = []


def _get_nc():
    if not _NC_CACHE:
        _NC_CACHE.append(build())
    return _NC_CACHE[0]


def make_in_maps(x_q, x_kv, wq, bq, wk, bk, wv, bv, wo, bo):
    bf = ml_dtypes.bfloat16
    in_maps = []
    bo_effs = []
    for c in range(NCORES):
        b, n = divmod(c, NH)
        hs = slice(n * HD, (n + 1) * HD)
        wq_h = wq[hs].astype(np.float64) * SCALE
        bo_eff = wo[:, hs].astype(np.float64) @ bv[hs].astype(np.float64)
        if n == 0:
            bo_eff = bo_eff + bo.astype(np.float64)
        bo_effs.append(bo_eff.astype(np.float32))
        in_maps.append({
            "xq": np.ascontiguousarray(
                x_q[b].reshape(C, S).reshape(2, P, S)).astype(bf),
            "xkv": np.ascontiguousarray(
                x_kv[b].reshape(C, S).reshape(2, P, S)).astype(bf),
            "wblob": np.ascontiguousarray(
                np.concatenate(
                    [np.concatenate([wk[hs].T, wv[hs].T], axis=1)
                     .reshape(2, P, P),
                     wq_h.T.reshape(2, P, HD)], axis=2)).astype(bf),
            "woT": np.ascontiguousarray(wo[:, hs].T).astype(bf),
            "bq": (bq[hs].astype(np.float64) * SCALE
                   ).astype(np.float32).reshape(HD, 1),
        })
    return in_maps, bo_effs


def assemble_output(results, bo_effs):
    # y_core is the unnormalized head partial; divide by the softmax
    # denominator and add the (host-folded) bias here.
    y = np.zeros((B, C, S), np.float32)
    for c in range(NCORES):
        b = c // NH
        den = results[c]["yden"].reshape(1, S)
        y[b] += results[c]["y"].astype(np.float32).reshape(C, S) / den \
            + bo_effs[c].reshape(C, 1)
    return y.reshape(B, C, HGT, WID)


def kernel(**inputs):
    nc = _get_nc()
    in_maps, bo_effs = make_in_maps(**inputs)
    res = run_bass_kernel_spmd(nc, in_maps, list(range(NCORES)))
    return assemble_output(res.results, bo_effs)


if __name__ == "__main__":
    nc = build()
    print("built + compiled ok")
